# revision 1
# baseline (speedup 1.0000x reference)
"""GNN message-passing layer (EquivariantMPLayer) on 8 Trainium2 NeuronCores.

Sharding: edges are sharded by destination-node range (dst // (N/8)) so each
core aggregates its own node range locally -- no collectives needed. Per core,
edges are sorted by dst and grouped into 128-node sub-windows; each window's
edge list is split into two streams by src < N/2 (dma_gather indices are
int16, so each gather table must have < 32768 rows) and padded to 128-edge
blocks. Per-window per-stream block counts are equalized across cores (max
over cores) so a single SPMD program serves all 8 cores.

Device pipeline per 128-edge block:
  - dma_gather of x[src] (from the lo/hi half table) and x[dst] (from the
    core-local node table), batched ~8k rows per call
  - PE transposes to feature-major
  - L1 matmul (mw1, K split 128 + 16 for host-precomputed RBF features)
  - Silu (ACT, fused bias, PSUM->SBUF)
  - L2 matmul -> per-edge messages (edge-major)
  - one-hot (DVE iota-compare vs window-relative dst) + scatter-matmul
    accumulating aggT[64, 128-node window] in PSUM
  - stream-lo: window partials parked in SBUF; stream-hi: combined flush
    with host-precomputed 1/max(cnt,1) and mb2*(cnt>0)
Then an update MLP + LayerNorm over the core's nodes, written row-major.
"""

import numpy as np

N = 50000
E = 800000
DIN = 64
DOUT = 64
NB = 16
MAX_RADIUS = 10.0
NCORES = 8
P = 128
GB = 16  # gather batch, in 128-edge blocks

_prog_cache = {}


def _wrap_idx16(arr):
    """[nblocks, 128] int idx -> [128, nblocks*8] int16 (16-wrapped, x8)."""
    nb = arr.shape[0]
    t16 = np.transpose(arr.reshape(nb, 8, 16), (2, 0, 1)).reshape(16, nb * 8)
    return np.tile(t16.astype(np.int16), (8, 1)).copy()


# ---------------------------------------------------------------------------
# Host-side structure / metadata
# ---------------------------------------------------------------------------

def _build_host_data(x, edge_index, edge_len, mw1, mb1, mw2, mb2,
                     uw1, ub1, uw2, ub2, ln_g, ln_b,
                     n=N, ncores=NCORES):
    nloc = n // ncores
    nhalf = (n + 1) // 2
    nw = (nloc + P - 1) // P
    npad = nw * P

    src = np.asarray(edge_index[0], dtype=np.int64)
    dst = np.asarray(edge_index[1], dtype=np.int64)
    x = np.asarray(x, dtype=np.float32)
    el = np.asarray(edge_len, dtype=np.float32)[:, 0]

    centers = np.linspace(0.0, MAX_RADIUS, NB, dtype=np.float64)
    width = (centers[1] - centers[0]) * 0.5
    rbf_all = np.exp(-((el[:, None].astype(np.float64) - centers) ** 2)
                     / (2.0 * width ** 2)).astype(np.float32)  # [E, 16]

    core_of = dst // nloc
    per_core = []
    cnt_s = np.zeros((2, ncores, nw), dtype=np.int64)
    for c in range(ncores):
        eids = np.nonzero(core_of == c)[0]
        dloc = (dst[eids] - c * nloc).astype(np.int64)
        order = np.argsort(dloc, kind="stable")
        eids = eids[order]
        dloc = dloc[order]
        w_of = dloc // P
        hi = (src[eids] >= nhalf).astype(np.int64)
        for s in (0, 1):
            cnt_s[s, c] = np.bincount(w_of[hi == s], minlength=nw)
        per_core.append((eids, dloc, w_of, hi))

    # per-(stream, window) block counts, equalized across cores
    bws = np.maximum(1, (cnt_s.max(axis=1) + P - 1) // P)  # [2, nw]
    # pad each stream's total to a multiple of 4 (supertile granularity)
    for s in (0, 1):
        bws[s, -1] += (-int(bws[s].sum())) % 4
    b_lo = int(bws[0].sum())
    b_hi = int(bws[1].sum())
    btot = b_lo + b_hi
    epad = btot * P

    # global block list: stream-lo blocks (windows in order), then stream-hi
    block_window = []
    block_stream = []
    for s in (0, 1):
        for w in range(nw):
            block_window += [w] * int(bws[s, w])
            block_stream += [s] * int(bws[s, w])
    block_window = np.array(block_window)
    block_stream = np.array(block_stream)
    # block offsets per (stream, window)
    boff = {}
    pos = 0
    for s in (0, 1):
        for w in range(nw):
            boff[(s, w)] = pos
            pos += int(bws[s, w])

    in_maps = []
    for c in range(ncores):
        eids, dloc, w_of, hi = per_core[c]
        sidx = np.zeros((btot, P), dtype=np.int64)
        didx = np.zeros((btot, P), dtype=np.int64)
        dwrel_arr = np.full((btot, P), 999.0, dtype=np.float32)
        rbf_arr = np.zeros((btot, P, NB), dtype=np.float32)
        for s in (0, 1):
            for w in range(nw):
                sel = np.nonzero((w_of == w) & (hi == s))[0]
                k = len(sel)
                if not k:
                    continue
                ee = eids[sel]
                b0 = boff[(s, w)]
                bidx = b0 + np.arange(k) // P
                pidx = np.arange(k) % P
                sidx[bidx, pidx] = src[ee] - s * nhalf
                didx[bidx, pidx] = dloc[sel]
                dwrel_arr[bidx, pidx] = (dloc[sel] - w * P).astype(np.float32)
                rbf_arr[bidx, pidx] = rbf_all[ee]

        cnt_n = np.zeros(npad, dtype=np.float32)
        cnt_n[:nloc] = np.bincount(dloc, minlength=nloc).astype(np.float32)
        inv = 1.0 / np.maximum(cnt_n, 1.0)
        has = (cnt_n > 0).astype(np.float32)
        # per-edge 1/max(cnt,1) of the edge's dst node (folded into one-hot)
        invE_arr = np.ones((btot, P), dtype=np.float32)
        for s in (0, 1):
            for w in range(nw):
                sel = np.nonzero((w_of == w) & (hi == s))[0]
                k = len(sel)
                if not k:
                    continue
                b0 = boff[(s, w)]
                invE_arr[b0 + np.arange(k) // P, np.arange(k) % P] = \
                    inv[dloc[sel]]

        import ml_dtypes as _mld
        xt_loc = np.zeros((DIN, npad), dtype=_mld.bfloat16)
        xt_loc[:, :nloc] = x[c * nloc:(c + 1) * nloc].T.astype(_mld.bfloat16)

        def padbf(a):
            out = np.zeros((a.shape[0], 2 * DIN), dtype=np.float16)
            out = out.astype(np.dtype('bfloat16') if hasattr(np, 'bfloat16')
                             else out.dtype)
            import ml_dtypes
            out = np.zeros((a.shape[0], 2 * DIN), dtype=ml_dtypes.bfloat16)
            out[:, :DIN] = a.astype(ml_dtypes.bfloat16)
            return out

        import ml_dtypes
        bf16 = ml_dtypes.bfloat16
        m = {
            "xa": padbf(x[:nhalf]),
            "xb": padbf(x[nhalf:]),
            "xloc": padbf(x[c * nloc:(c + 1) * nloc]),
            "sidx_lo": _wrap_idx16(sidx[:b_lo]),
            "sidx_hi": _wrap_idx16(sidx[b_lo:]),
            "didx": _wrap_idx16(didx),
            "dwrelT": dwrel_arr.T.copy(),                    # [128, btot] f32
            "invT": invE_arr.T.copy(),                       # [128, btot] f32
            "rbfT": rbf_arr.reshape(epad, NB).T.astype(bf16),  # [16, epad]
            "xTloc": xt_loc,
            "hasrow": has.reshape(1, npad).astype(bf16),
            "mb2row": np.asarray(mb2, np.float32).reshape(1, DOUT)
                        .astype(bf16),
            "mw1_sd": np.asarray(mw1, np.float32)[:2 * DIN].astype(bf16),
            "mw1_r": np.asarray(mw1, np.float32)[2 * DIN:].astype(bf16),
            "mb1": np.asarray(mb1, np.float32).reshape(2 * DOUT, 1).copy(),
            "mw2": np.asarray(mw2, np.float32).astype(bf16),
            # upd layout is [agg; x] -> swap uw1 row blocks to match
            "uw1": np.concatenate([np.asarray(uw1, np.float32)[DIN:],
                                   np.asarray(uw1, np.float32)[:DIN]],
                                  axis=0).astype(bf16),
            "ub1": np.asarray(ub1, np.float32).reshape(DOUT, 1).copy(),
            "uw2": np.asarray(uw2, np.float32).astype(bf16),
            "ub2": np.asarray(ub2, np.float32).reshape(DOUT, 1).copy(),
            "lng": np.broadcast_to(np.asarray(ln_g, np.float32)[None, :],
                                   (P, DOUT)).copy(),
            "lnb": np.broadcast_to(np.asarray(ln_b, np.float32)[None, :],
                                   (P, DOUT)).copy(),
            "iota": np.broadcast_to(np.arange(P).astype(bf16)[None, :],
                                    (P, P)).copy(),
            "ident": np.eye(P, dtype=np.float32).astype(bf16),
            "identf": np.eye(P, dtype=np.float32),
        }
        in_maps.append(m)

    struct = dict(n=n, nhalf=nhalf, nloc=nloc, nw=nw, npad=npad,
                  b_lo=b_lo, b_hi=b_hi, btot=btot, epad=epad,
                  bws=tuple(tuple(int(v) for v in row) for row in bws),
                  block_window=tuple(int(v) for v in block_window),
                  block_stream=tuple(int(v) for v in block_stream))
    return struct, in_maps


# ---------------------------------------------------------------------------
# Device program
# ---------------------------------------------------------------------------

def _build_program(struct, use_silu=True, debug_dump=False,
                   inline_upd=True):
    import concourse.bass as bass
    import concourse.mybir as mybir
    import concourse.tile as tile
    from concourse import bacc

    f32 = mybir.dt.float32
    bf = mybir.dt.bfloat16
    i16 = mybir.dt.int16
    n, nhalf, nloc, nw, npad = (struct["n"], struct["nhalf"], struct["nloc"],
                                struct["nw"], struct["npad"])
    b_lo, b_hi, btot, epad = (struct["b_lo"], struct["b_hi"],
                              struct["btot"], struct["epad"])
    block_window = struct["block_window"]
    block_stream = struct["block_stream"]

    # first/last block of each (stream, window)
    wfirst = {}
    wlast = {}
    for g, (w, s) in enumerate(zip(block_window, block_stream)):
        wfirst.setdefault((s, w), g)
        wlast[(s, w)] = g

    nc = bacc.Bacc("TRN2", target_bir_lowering=False, debug=False,
                   enable_asserts=False, num_devices=NCORES,
                   num_swdge_queues=4)

    xa_d = nc.dram_tensor("xa", [nhalf, 2 * DIN], bf, kind="ExternalInput")
    xb_d = nc.dram_tensor("xb", [n - nhalf, 2 * DIN], bf,
                          kind="ExternalInput")
    xloc_d = nc.dram_tensor("xloc", [nloc, 2 * DIN], bf, kind="ExternalInput")
    sidx_lo_d = nc.dram_tensor("sidx_lo", [P, b_lo * 8], i16,
                               kind="ExternalInput")
    sidx_hi_d = nc.dram_tensor("sidx_hi", [P, b_hi * 8], i16,
                               kind="ExternalInput")
    didx_d = nc.dram_tensor("didx", [P, btot * 8], i16, kind="ExternalInput")
    dwrelT_d = nc.dram_tensor("dwrelT", [P, btot], f32, kind="ExternalInput")
    invT_d = nc.dram_tensor("invT", [P, btot], f32, kind="ExternalInput")
    rbfT_d = nc.dram_tensor("rbfT", [NB, epad], bf, kind="ExternalInput")
    xTloc_d = nc.dram_tensor("xTloc", [DIN, npad], bf, kind="ExternalInput")
    hasrow_d = nc.dram_tensor("hasrow", [1, npad], bf, kind="ExternalInput")
    mb2row_d = nc.dram_tensor("mb2row", [1, DOUT], bf, kind="ExternalInput")
    mw1_sd_d = nc.dram_tensor("mw1_sd", [2 * DIN, 2 * DOUT], bf,
                              kind="ExternalInput")
    mw1_r_d = nc.dram_tensor("mw1_r", [NB, 2 * DOUT], bf,
                             kind="ExternalInput")
    mb1_d = nc.dram_tensor("mb1", [2 * DOUT, 1], f32, kind="ExternalInput")
    mw2_d = nc.dram_tensor("mw2", [2 * DOUT, DOUT], bf, kind="ExternalInput")
    uw1_d = nc.dram_tensor("uw1", [DIN + DOUT, DOUT], bf,
                           kind="ExternalInput")
    ub1_d = nc.dram_tensor("ub1", [DOUT, 1], f32, kind="ExternalInput")
    uw2_d = nc.dram_tensor("uw2", [DOUT, DOUT], bf, kind="ExternalInput")
    ub2_d = nc.dram_tensor("ub2", [DOUT, 1], f32, kind="ExternalInput")
    lng_d = nc.dram_tensor("lng", [P, DOUT], f32, kind="ExternalInput")
    lnb_d = nc.dram_tensor("lnb", [P, DOUT], f32, kind="ExternalInput")
    iota_d = nc.dram_tensor("iota", [P, P], bf, kind="ExternalInput")
    ident_d = nc.dram_tensor("ident", [P, P], bf, kind="ExternalInput")
    identf_d = nc.dram_tensor("identf", [P, P], f32, kind="ExternalInput")
    out_d = nc.dram_tensor("out", [npad, DOUT], f32, kind="ExternalOutput")
    if debug_dump:
        dbg_xT_d = nc.dram_tensor("dbg_xT", [P, 512], f32,
                                  kind="ExternalOutput")
        dbg_hT_d = nc.dram_tensor("dbg_hT", [P, 512], f32,
                                  kind="ExternalOutput")
        dbg_msg_d = nc.dram_tensor("dbg_msg", [P, 4 * DOUT], f32,
                                   kind="ExternalOutput")
        dbg_oh_d = nc.dram_tensor("dbg_oh", [P, P], f32,
                                  kind="ExternalOutput")
        dbg_upd_d = nc.dram_tensor("dbg_upd", [P, npad], f32,
                                   kind="ExternalOutput")

    AX = mybir.AxisListType
    OP = mybir.AluOpType
    ACT = mybir.ActivationFunctionType

    with tile.TileContext(nc) as tc:
        with (
            tc.tile_pool(name="const", bufs=1) as cpool,
            tc.tile_pool(name="gath", bufs=6) as gpool,
            tc.tile_pool(name="work", bufs=4) as wpool,
            tc.tile_pool(name="oh", bufs=12) as opool,
            tc.tile_pool(name="pt", bufs=2, space="PSUM") as pt_pool,
            tc.tile_pool(name="ph", bufs=2, space="PSUM") as ph_pool,
            tc.tile_pool(name="pm", bufs=2, space="PSUM") as pm_pool,
            tc.tile_pool(name="pa", bufs=2, space="PSUM") as pa_pool,
        ):
            def cload(dram, shape, dtype=f32):
                t = cpool.tile(shape, dtype, name=dram.name + "_t")
                nc.sync.dma_start(out=t[:], in_=dram[:])
                return t

            # small consts first (stream in ~2us), then gather tables,
            # then everything the message loop needs later
            iota_t = cload(iota_d, [P, P], bf)
            ident_t = cload(ident_d, [P, P], bf)
            mw1_sd_t = cload(mw1_sd_d, [2 * DIN, 2 * DOUT], bf)
            mw1_r_t = cload(mw1_r_d, [NB, 2 * DOUT], bf)
            mb1_t = cload(mb1_d, [2 * DOUT, 1])
            mw2_t = cload(mw2_d, [2 * DOUT, DOUT], bf)
            mb2row_t = cload(mb2row_d, [1, DOUT], bf)
            sidx_lo_t = cload(sidx_lo_d, [P, b_lo * 8], i16)
            didx_t = cload(didx_d, [P, btot * 8], i16)
            dwrelT_t = cload(dwrelT_d, [P, btot])
            invT_t = cload(invT_d, [P, btot])
            sidx_hi_t = cload(sidx_hi_d, [P, b_hi * 8], i16)
            hasrow_t = cload(hasrow_d, [1, npad], bf)
            identf_t = cload(identf_d, [P, P])
            uw1_t = cload(uw1_d, [DIN + DOUT, DOUT], bf)
            ub1_t = cload(ub1_d, [DOUT, 1])
            uw2_t = cload(uw2_d, [DOUT, DOUT], bf)
            ub2_t = cload(ub2_d, [DOUT, 1])
            lng_t = cload(lng_d, [P, DOUT])
            lnb_t = cload(lnb_d, [P, DOUT])

            eps_t = cpool.tile([P, 1], f32, name="eps_t")
            nc.vector.memset(eps_t[:], 1e-5)

            # stream-lo window partials (bf16: re-injected via PE matmul)
            agglo_t = cpool.tile([DOUT, npad], bf, name="agglo_t")
            # per-4-window update-MLP input chunks:
            # rows 0:64 = aggT, rows 64:128 = xT
            UT = 512
            nchunk = (npad + UT - 1) // UT
            upd_c = []
            for k in range(nchunk):
                cw = min(UT, npad - k * UT)
                t = cpool.tile([P, cw], bf, name=f"upd_c{k}")
                nc.sync.dma_start(out=t[DOUT:P, :],
                                  in_=xTloc_d[:, k * UT:k * UT + cw])
                upd_c.append(t)

            # LN intermediates parked per chunk; sqrt batched at the end
            zc_all = [cpool.tile([P, 4 * DOUT], f32, name=f"zc_all{k}")
                      for k in range(nchunk)]
            red2_all = cpool.tile([P, 4 * nchunk], f32, name="red2_all")

            pa_cur = {}
            upd_done = [False] * nchunk

            def emit_upd(k):
                # ---- update MLP + LayerNorm for node chunk k ----
                upd_done[k] = True
                u0 = k * UT
                cw = min(UT, npad - u0)
                nj = cw // P
                upd_in = upd_c[k]
                pu = ph_pool.tile([P, 512], f32, tag="ph", name=f"pu_{u0}")
                nc.tensor.matmul(pu[0:DOUT, 0:cw], uw1_t[:],
                                 upd_in[:, 0:cw], start=True, stop=True)
                uh_sb = wpool.tile([DOUT, UT], bf, tag="uh", name=f"uh_{u0}")
                if use_silu:
                    nc.scalar.activation(out=uh_sb[:, 0:cw],
                                         in_=pu[0:DOUT, 0:cw],
                                         func=ACT.Silu, bias=ub1_t[:, 0:1])
                else:
                    sg2 = wpool.tile([DOUT, UT], bf, tag="sg2",
                                     name=f"sg2_{u0}")
                    nc.scalar.activation(out=sg2[:, 0:cw],
                                         in_=pu[0:DOUT, 0:cw],
                                         func=ACT.Sigmoid, bias=ub1_t[:, 0:1])
                    nc.scalar.activation(out=uh_sb[:, 0:cw],
                                         in_=pu[0:DOUT, 0:cw],
                                         func=ACT.Identity, bias=ub1_t[:, 0:1])
                    nc.vector.tensor_tensor(out=uh_sb[:, 0:cw],
                                            in0=uh_sb[:, 0:cw],
                                            in1=sg2[:, 0:cw], op=OP.mult)
                pz = pt_pool.tile([P, 512], f32, tag="pxT", name=f"pz_{u0}")
                nc.tensor.matmul(pz[0:DOUT, 0:cw], uw2_t[:], uh_sb[:, 0:cw],
                                 start=True, stop=True)
                zT_sb = wpool.tile([DOUT, UT], bf, tag="zT", name=f"zT_{u0}")
                nc.scalar.activation(out=zT_sb[:, 0:cw], in_=pz[0:DOUT, 0:cw],
                                     func=ACT.Identity, bias=ub2_t[:, 0:1])

                pz2 = pm_pool.tile([P, 4 * DOUT], bf, tag="pm",
                                   name=f"pz2_{u0}")
                for j in range(nj):
                    nc.tensor.transpose(
                        out=pz2[:, j * DOUT:(j + 1) * DOUT],
                        in_=zT_sb[:, j * P:(j + 1) * P],
                        identity=ident_t[0:DOUT, 0:DOUT])
                # LN phase A: mean-center + variance sum; sqrt deferred
                zc = zc_all[k]
                red = wpool.tile([P, 4], f32, tag="red", name=f"red_{u0}")
                z3 = pz2[:, 0:nj * DOUT].rearrange("p (j d) -> p j d", d=DOUT)
                nc.vector.tensor_reduce(out=red[:, 0:nj], in_=z3, axis=AX.X,
                                        op=OP.add)
                nc.vector.tensor_scalar_mul(red[:, 0:nj], red[:, 0:nj],
                                            -1.0 / DOUT)
                zc3 = zc[:, 0:nj * DOUT].rearrange("p (j d) -> p j d", d=DOUT)
                nc.vector.tensor_tensor(
                    out=zc3, in0=z3,
                    in1=red[:, 0:nj, None].to_broadcast([P, nj, DOUT]),
                    op=OP.add)
                sq = wpool.tile([P, 4 * DOUT], f32, tag="sq", name=f"sq_{u0}")
                sq3 = sq[:, 0:nj * DOUT].rearrange("p (j d) -> p j d", d=DOUT)
                nc.vector.tensor_tensor(out=sq3, in0=zc3, in1=zc3, op=OP.mult)
                nc.vector.tensor_reduce(out=red2_all[:, 4 * k:4 * k + nj],
                                        in_=sq3, axis=AX.X, op=OP.add)

            def emit_ln_final():
                # batched sqrt + reciprocal, then scale/affine/store per chunk
                sd = cpool.tile([P, 4 * nchunk], f32, name="sd_all")
                nc.scalar.activation(out=sd[:], in_=red2_all[:],
                                     func=ACT.Sqrt, scale=1.0 / DOUT,
                                     bias=eps_t[:, 0:1])
                rs = cpool.tile([P, 4 * nchunk], f32, name="rs_all")
                nc.vector.reciprocal(out=rs[:], in_=sd[:])
                for k in range(nchunk):
                    u0 = k * UT
                    cw = min(UT, npad - u0)
                    nj = cw // P
                    zc = zc_all[k]
                    zc3 = zc[:, 0:nj * DOUT].rearrange("p (j d) -> p j d",
                                                       d=DOUT)
                    zn = wpool.tile([P, 4 * DOUT], f32, tag="zn",
                                    name=f"zn_{u0}")
                    zn3 = zn[:, 0:nj * DOUT].rearrange("p (j d) -> p j d",
                                                       d=DOUT)
                    nc.vector.tensor_tensor(
                        out=zn3, in0=zc3,
                        in1=rs[:, 4 * k:4 * k + nj, None]
                            .to_broadcast([P, nj, DOUT]),
                        op=OP.mult)
                    for j in range(nj):
                        js = slice(j * DOUT, (j + 1) * DOUT)
                        nc.vector.tensor_tensor(out=zn[:, js], in0=zn[:, js],
                                                in1=lng_t[:], op=OP.mult)
                        nc.vector.tensor_tensor(out=zn[:, js], in0=zn[:, js],
                                                in1=lnb_t[:], op=OP.add)
                    od = out_d[u0:u0 + cw, :].rearrange(
                        "(j p) d -> p j d", p=P)
                    zn3o = zn[:, 0:nj * DOUT].rearrange(
                        "p (j d) -> p j d", d=DOUT)
                    nc.sync.dma_start(out=od, in_=zn3o)

            def do_stream(s, g0s, g1s, src_tab, sidx_t, sidx_goff):
                for b0 in range(g0s, g1s, GB):
                    gb = min(GB, g1s - b0)
                    xg = gpool.tile([P, 2, GB, 2 * DIN], bf,
                                    name=f"xg_{s}_{b0}", tag="xg")
                    c0 = (b0 - sidx_goff) * 8
                    # split each gather across 2 queues (4 concurrent
                    # descriptor streams per batch)
                    gh = gb // 2
                    if gh == 0:
                        nc.gpsimd.dma_gather(
                            out_ap=xg[:, 0, 0:gb, :], in_ap=src_tab,
                            idxs_ap=sidx_t[:, c0:c0 + gb * 8],
                            num_idxs=gb * P, num_idxs_reg=gb * P,
                            elem_size=2 * DIN, single_packet=True,
                            queue_num=0)
                        nc.gpsimd.dma_gather(
                            out_ap=xg[:, 1, 0:gb, :], in_ap=xloc_d[:],
                            idxs_ap=didx_t[:, b0 * 8:(b0 + gb) * 8],
                            num_idxs=gb * P, num_idxs_reg=gb * P,
                            elem_size=2 * DIN, single_packet=True,
                            queue_num=2)
                    else:
                        for h, (lo, hi) in enumerate(((0, gh), (gh, gb))):
                            nh = hi - lo
                            nc.gpsimd.dma_gather(
                                out_ap=xg[:, 0, lo:hi, :], in_ap=src_tab,
                                idxs_ap=sidx_t[:, c0 + lo * 8:c0 + hi * 8],
                                num_idxs=nh * P, num_idxs_reg=nh * P,
                                elem_size=2 * DIN, single_packet=True,
                                queue_num=h)
                            nc.gpsimd.dma_gather(
                                out_ap=xg[:, 1, lo:hi, :], in_ap=xloc_d[:],
                                idxs_ap=didx_t[:, (b0 + lo) * 8:(b0 + hi) * 8],
                                num_idxs=nh * P, num_idxs_reg=nh * P,
                                elem_size=2 * DIN, single_packet=True,
                                queue_num=2 + h)

                    for st0 in range(0, gb, 4):
                        st_blocks = [b0 + st0 + j for j in range(4)]
                        dbg_here = debug_dump and st_blocks[0] == 0

                        pxT = pt_pool.tile([P, 512], bf, tag="pxT",
                                           name=f"pxT_{st_blocks[0]}")
                        xg_flat = xg[:].rearrange("p a g d -> p (a g d)")
                        RW = 2 * DIN  # padded row width
                        for j in range(4):
                            gj = st0 + j
                            # full [128,128] transpose of [zeropad|dst_j]
                            # (64 cols before a dst row are the previous
                            # row's zero padding), then overwrite
                            # partitions 0:64 with src_j.T
                            o1 = (GB + gj) * RW - DIN
                            nc.tensor.transpose(
                                out=pxT[:, j * P:(j + 1) * P],
                                in_=xg_flat[:, o1:o1 + 2 * DIN],
                                identity=ident_t[:])
                            nc.tensor.transpose(
                                out=pxT[0:DIN, j * P:(j + 1) * P],
                                in_=xg_flat[:, gj * RW:gj * RW + DIN],
                                identity=ident_t[:])
                        xT_sb = wpool.tile([P, 512], bf, tag="xT",
                                           name=f"xT_{st_blocks[0]}")
                        nc.scalar.copy(out=xT_sb[:], in_=pxT[:])
                        if dbg_here:
                            nc.sync.dma_start(out=dbg_xT_d[:], in_=xT_sb[:])

                        rbf_t = wpool.tile([NB, 512], bf, tag="rbf",
                                           name=f"rbf_{st_blocks[0]}")
                        e0 = st_blocks[0] * P
                        nc.sync.dma_start(out=rbf_t[:],
                                          in_=rbfT_d[:, e0:e0 + 512])

                        ph = ph_pool.tile([P, 512], f32, tag="ph",
                                          name=f"ph_{st_blocks[0]}")
                        nc.tensor.matmul(ph[:], mw1_sd_t[:], xT_sb[:],
                                         start=True, stop=False)
                        nc.tensor.matmul(ph[:], mw1_r_t[:], rbf_t[:],
                                         start=False, stop=True)

                        hT_sb = wpool.tile([P, 512], bf, tag="hT",
                                           name=f"hT_{st_blocks[0]}")
                        if use_silu:
                            nc.scalar.activation(out=hT_sb[:], in_=ph[:],
                                                 func=ACT.Silu,
                                                 bias=mb1_t[:, 0:1])
                        else:
                            sg = wpool.tile([P, 512], bf, tag="sg",
                                            name=f"sg_{st_blocks[0]}")
                            nc.scalar.activation(out=sg[:], in_=ph[:],
                                                 func=ACT.Sigmoid,
                                                 bias=mb1_t[:, 0:1])
                            nc.scalar.activation(out=hT_sb[:], in_=ph[:],
                                                 func=ACT.Identity,
                                                 bias=mb1_t[:, 0:1])
                            nc.vector.tensor_tensor(out=hT_sb[:],
                                                    in0=hT_sb[:],
                                                    in1=sg[:], op=OP.mult)
                        if dbg_here:
                            nc.sync.dma_start(out=dbg_hT_d[:], in_=hT_sb[:])

                        pm = pm_pool.tile([P, 4 * DOUT], f32, tag="pm",
                                          name=f"pm_{st_blocks[0]}")
                        for j in range(4):
                            nc.tensor.matmul(pm[:, j * DOUT:(j + 1) * DOUT],
                                             hT_sb[:, j * P:(j + 1) * P],
                                             mw2_t[:], start=True, stop=True)
                        msg_sb = wpool.tile([P, 4 * DOUT], bf, tag="msg",
                                            name=f"msg_{st_blocks[0]}")
                        nc.scalar.copy(out=msg_sb[:], in_=pm[:])
                        if dbg_here:
                            nc.sync.dma_start(out=dbg_msg_d[:], in_=msg_sb[:])

                        for j in range(4):
                            g = st_blocks[j]
                            w = block_window[g]
                            oh = opool.tile([P, P], bf, tag="oh",
                                            name=f"oh_{g}")
                            # inv-scaled one-hot: inv[dst[e]] at col dwrel[e]
                            nc.any.tensor_scalar(
                                out=oh[:], in0=iota_t[:],
                                scalar1=dwrelT_t[:, g:g + 1],
                                scalar2=invT_t[:, g:g + 1],
                                op0=OP.is_equal, op1=OP.mult)
                            if dbg_here and j == 0:
                                nc.sync.dma_start(out=dbg_oh_d[:], in_=oh[:])
                            wc = slice(w * P, (w + 1) * P)
                            if g == wfirst[(s, w)]:
                                pa_cur[w] = pa_pool.tile(
                                    [DOUT, P], f32, tag="pa",
                                    name=f"pa_s{s}_w{w}")
                                if s == 1:
                                    # seed hi accumulation with the lo
                                    # partial and mb2*(cnt>0), via PE
                                    nc.tensor.matmul(
                                        pa_cur[w][:],
                                        ident_t[0:DOUT, 0:DOUT],
                                        agglo_t[:, wc],
                                        start=True, stop=False,
                                        skip_group_check=True)
                                    nc.tensor.matmul(
                                        pa_cur[w][:],
                                        mb2row_t[:], hasrow_t[:, wc],
                                        start=False, stop=False,
                                        skip_group_check=True)
                            nc.tensor.matmul(
                                pa_cur[w][:],
                                msg_sb[:, j * DOUT:(j + 1) * DOUT], oh[:],
                                start=(s == 0 and g == wfirst[(s, w)]),
                                stop=(g == wlast[(s, w)]),
                                skip_group_check=True)
                            if g != wlast[(s, w)]:
                                continue
                            if s == 0:
                                nc.scalar.copy(out=agglo_t[:, wc],
                                               in_=pa_cur[w][:])
                            else:
                                kc = w // 4
                                uc = slice((w % 4) * P, (w % 4 + 1) * P)
                                nc.scalar.copy(out=upd_c[kc][0:DOUT, uc],
                                               in_=pa_cur[w][:])
                            del pa_cur[w]
                            if (inline_upd and s == 1
                                    and w == min(4 * (w // 4) + 4, nw) - 1):
                                emit_upd(w // 4)

            do_stream(0, 0, b_lo, xa_d[:], sidx_lo_t[:], 0)
            do_stream(1, b_lo, btot, xb_d[:], sidx_hi_t[:], b_lo)

            # any update chunks not emitted inline
            for k in range(nchunk):
                if not upd_done[k]:
                    emit_upd(k)
            emit_ln_final()

    nc.compile()
    return nc


# ---------------------------------------------------------------------------
# Entry point
# ---------------------------------------------------------------------------

last_results = None


def kernel(x, edge_index, edge_vec, edge_len,
           mw1, mb1, mw2, mb2, uw1, ub1, uw2, ub2, ln_g, ln_b):
    global last_results
    import os
    from concourse.bass_utils import run_bass_kernel_spmd

    struct, in_maps = _build_host_data(
        x, edge_index, edge_len, mw1, mb1, mw2, mb2,
        uw1, ub1, uw2, ub2, ln_g, ln_b)

    key = (struct["n"], struct["btot"], struct["bws"])
    if key not in _prog_cache:
        _prog_cache[key] = _build_program(
            struct, use_silu=os.environ.get("K_NO_SILU", "") == "",
            inline_upd=os.environ.get("K_INLINE_UPD", "") != "")
    nc = _prog_cache[key]

    kw = {}
    if os.environ.get("K_TRACE", ""):
        try:
            import profile_shim
            profile_shim.install()
        except ImportError:
            pass
        kw = dict(trace=True, trace_cores=list(range(NCORES)),
                  tmpdir="/tmp/ntff_out")
    res = run_bass_kernel_spmd(nc, in_maps, core_ids=list(range(NCORES)), **kw)
    last_results = res
    nloc = struct["nloc"]
    out = np.concatenate([res.results[c]["out"][:nloc] for c in range(NCORES)],
                         axis=0)
    return out.astype(np.float32)



# revision 3
# speedup vs baseline: 2.3222x; 2.3222x over previous
"""GNN message-passing layer (EquivariantMPLayer) on 8 Trainium2 NeuronCores.

Sharding: edges are sharded by destination-node range (dst // (N/8)) so each
core aggregates its own node range locally -- no collectives needed.

Host prep does the gather: for each core's dst-sorted edge list, the host
builds a feature-major bf16 stream vT[128, epad] where each edge column is
v = [x[src]; x[dst]] + M @ rbf, with M = (mw1_sd^T)^{-1} @ mw1_r^T. Since
mw1_sd is square and invertible, mw1_sd^T @ v == mw1_sd^T @ [xs;xd] +
mw1_r^T @ rbf exactly, so the RBF term rides along in the same 128-row
matmul and the device does no gathers, no transposes and no rbf matmul.

Device pipeline per 4-block supertile (512 edges):
  - one sequential DMA of vT columns (128 KB)
  - one DVE op builds 4 one-hot scatter blocks: oh[e, n] = (iota == dwrel)
  - L1 matmul (mw1_sd stationary, vT moving) -> ph[128 hd, 512] PSUM
  - Silu (ACT, fused mb1 bias) -> hT bf16
  - L2 per block: lhsT=hT block -> msg edge-major [128 e, 64] PSUM -> bf16
  - scatter per block: lhsT=oh, rhs=msg -> S[node, dout] PSUM accumulated
    over the window's blocks
  - window flush: DVE inv-scale (per-node 1/max(cnt,1)), PE transpose to
    [dout, node], += mb2 (x) hasrow via K=1 matmul, copy into update chunk
Then an update MLP + LayerNorm over the core's nodes, written row-major.
"""

import numpy as np

N = 50000
E = 800000
DIN = 64
DOUT = 64
NB = 16
MAX_RADIUS = 10.0
NCORES = 8
P = 128

_prog_cache = {}


# ---------------------------------------------------------------------------
# Host-side structure / metadata
# ---------------------------------------------------------------------------

def _build_host_data(x, edge_index, edge_len, mw1, mb1, mw2, mb2,
                     uw1, ub1, uw2, ub2, ln_g, ln_b,
                     n=N, ncores=NCORES):
    import ml_dtypes
    bf16 = ml_dtypes.bfloat16

    nloc = n // ncores
    nw = (nloc + P - 1) // P
    npad = nw * P

    src = np.asarray(edge_index[0], dtype=np.int64)
    dst = np.asarray(edge_index[1], dtype=np.int64)
    x = np.asarray(x, dtype=np.float32)
    el = np.asarray(edge_len, dtype=np.float32)[:, 0]

    centers = np.linspace(0.0, MAX_RADIUS, NB, dtype=np.float64)
    width = (centers[1] - centers[0]) * 0.5
    rbf_all = np.exp(-((el[:, None].astype(np.float64) - centers) ** 2)
                     / (2.0 * width ** 2)).astype(np.float32)  # [E, 16]

    # fold mw1_r into the shipped edge vectors:
    # v = [xs; xd] + M @ rbf with M = (mw1_sd_bf^T)^-1 @ mw1_r^T (f64 solve
    # against the bf16-rounded mw1_sd actually used on device)
    mw1 = np.asarray(mw1, np.float32)
    mw1_sd_bf = mw1[:2 * DIN].astype(bf16)
    mw1_r = mw1[2 * DIN:]
    M = np.linalg.solve(mw1_sd_bf.astype(np.float64).T,
                        mw1_r.astype(np.float64).T)  # [128, 16]
    Mt = M.T.astype(np.float32)  # [16, 128]

    core_of = dst // nloc
    per_core = []
    cnt_cw = np.zeros((ncores, nw), dtype=np.int64)
    for c in range(ncores):
        eids = np.nonzero(core_of == c)[0]
        dloc = (dst[eids] - c * nloc).astype(np.int64)
        order = np.argsort(dloc, kind="stable")
        eids = eids[order]
        dloc = dloc[order]
        w_of = dloc // P
        cnt_cw[c] = np.bincount(w_of, minlength=nw)
        per_core.append((eids, dloc, w_of))

    # per-window block counts, equalized across cores; pad total to %4
    bws = np.maximum(1, (cnt_cw.max(axis=0) + P - 1) // P)  # [nw]
    bws[-1] += (-int(bws.sum())) % 4
    btot = int(bws.sum())
    epad = btot * P

    block_window = []
    for w in range(nw):
        block_window += [w] * int(bws[w])
    block_window = np.array(block_window)
    boff = np.concatenate([[0], np.cumsum(bws)])  # block offset per window

    in_maps = []
    for c in range(ncores):
        eids, dloc, w_of = per_core[c]
        ne = len(eids)
        # position of each edge inside its window's block range
        # edges are dst-sorted so within a window they are consecutive
        wstart = np.concatenate([[0], np.cumsum(cnt_cw[c])])
        pos_in_w = np.arange(ne) - wstart[w_of]
        slot = boff[w_of] * P + pos_in_w  # global padded slot per edge

        vpair = np.zeros((epad, 2 * DIN), dtype=np.float32)
        vpair[slot, :DIN] = x[src[eids]]
        vpair[slot, DIN:] = x[dst[eids]]
        vpair[slot] += rbf_all[eids] @ Mt
        vT = np.ascontiguousarray(vpair.T).astype(bf16)  # [128, epad]

        dwrelT = np.full((P, btot), 999.0, dtype=np.float32)
        dwrelT[pos_in_w % P, boff[w_of] + pos_in_w // P] = \
            (dloc - w_of * P).astype(np.float32)

        cnt_n = np.zeros(npad, dtype=np.float32)
        cnt_n[:nloc] = np.bincount(dloc, minlength=nloc).astype(np.float32)
        invN = np.ascontiguousarray(
            (1.0 / np.maximum(cnt_n, 1.0)).reshape(nw, P).T)  # [128, nw]
        has = (cnt_n > 0).astype(np.float32)

        xt_loc = np.zeros((DIN, npad), dtype=bf16)
        xt_loc[:, :nloc] = x[c * nloc:(c + 1) * nloc].T.astype(bf16)

        iota512 = np.broadcast_to(
            (np.arange(512) % P).astype(np.float32)[None, :], (P, 512)).copy()

        m = {
            "vT": vT,
            "dwrelT": dwrelT,
            "invN": invN,
            "xTloc": xt_loc,
            "hasrow": has.reshape(1, npad).astype(bf16),
            "mb2row": np.asarray(mb2, np.float32).reshape(1, DOUT)
                        .astype(bf16),
            "mw1_sd": mw1_sd_bf,
            "mb1": np.asarray(mb1, np.float32).reshape(2 * DOUT, 1).copy(),
            "mw2": np.asarray(mw2, np.float32).astype(bf16),
            # upd layout is [agg; x] -> swap uw1 row blocks to match
            "uw1": np.concatenate([np.asarray(uw1, np.float32)[DIN:],
                                   np.asarray(uw1, np.float32)[:DIN]],
                                  axis=0).astype(bf16),
            "ub1": np.asarray(ub1, np.float32).reshape(DOUT, 1).copy(),
            "uw2": np.asarray(uw2, np.float32).astype(bf16),
            "ub2": np.asarray(ub2, np.float32).reshape(DOUT, 1).copy(),
            "lng": np.broadcast_to(np.asarray(ln_g, np.float32)[None, :],
                                   (P, DOUT)).copy(),
            "lnb": np.broadcast_to(np.asarray(ln_b, np.float32)[None, :],
                                   (P, DOUT)).copy(),
            "iota512": iota512,
            "ident": np.eye(P, dtype=np.float32).astype(bf16),
        }
        in_maps.append(m)

    struct = dict(n=n, nloc=nloc, nw=nw, npad=npad, btot=btot, epad=epad,
                  bws=tuple(int(v) for v in bws),
                  block_window=tuple(int(v) for v in block_window))
    return struct, in_maps


# ---------------------------------------------------------------------------
# Device program
# ---------------------------------------------------------------------------

def _build_program(struct):
    import concourse.bass as bass
    import concourse.mybir as mybir
    import concourse.tile as tile
    from concourse import bacc

    f32 = mybir.dt.float32
    bf = mybir.dt.bfloat16
    n, nloc, nw, npad = (struct["n"], struct["nloc"], struct["nw"],
                         struct["npad"])
    btot, epad = struct["btot"], struct["epad"]
    block_window = struct["block_window"]

    wfirst = {}
    wlast = {}
    for g, w in enumerate(block_window):
        wfirst.setdefault(w, g)
        wlast[w] = g

    nc = bacc.Bacc("TRN2", target_bir_lowering=False, debug=False,
                   enable_asserts=False, num_devices=NCORES)

    vT_d = nc.dram_tensor("vT", [P, epad], bf, kind="ExternalInput")
    dwrelT_d = nc.dram_tensor("dwrelT", [P, btot], f32, kind="ExternalInput")
    invN_d = nc.dram_tensor("invN", [P, nw], f32, kind="ExternalInput")
    xTloc_d = nc.dram_tensor("xTloc", [DIN, npad], bf, kind="ExternalInput")
    hasrow_d = nc.dram_tensor("hasrow", [1, npad], bf, kind="ExternalInput")
    mb2row_d = nc.dram_tensor("mb2row", [1, DOUT], bf, kind="ExternalInput")
    mw1_sd_d = nc.dram_tensor("mw1_sd", [2 * DIN, 2 * DOUT], bf,
                              kind="ExternalInput")
    mb1_d = nc.dram_tensor("mb1", [2 * DOUT, 1], f32, kind="ExternalInput")
    mw2_d = nc.dram_tensor("mw2", [2 * DOUT, DOUT], bf, kind="ExternalInput")
    uw1_d = nc.dram_tensor("uw1", [DIN + DOUT, DOUT], bf,
                           kind="ExternalInput")
    ub1_d = nc.dram_tensor("ub1", [DOUT, 1], f32, kind="ExternalInput")
    uw2_d = nc.dram_tensor("uw2", [DOUT, DOUT], bf, kind="ExternalInput")
    ub2_d = nc.dram_tensor("ub2", [DOUT, 1], f32, kind="ExternalInput")
    lng_d = nc.dram_tensor("lng", [P, DOUT], f32, kind="ExternalInput")
    lnb_d = nc.dram_tensor("lnb", [P, DOUT], f32, kind="ExternalInput")
    iota512_d = nc.dram_tensor("iota512", [P, 512], f32,
                               kind="ExternalInput")
    ident_d = nc.dram_tensor("ident", [P, P], bf, kind="ExternalInput")
    out_d = nc.dram_tensor("out", [npad, DOUT], f32, kind="ExternalOutput")

    AX = mybir.AxisListType
    OP = mybir.AluOpType
    ACT = mybir.ActivationFunctionType

    with tile.TileContext(nc) as tc:
        with (
            tc.tile_pool(name="const", bufs=1) as cpool,
            tc.tile_pool(name="gath", bufs=4) as gpool,
            tc.tile_pool(name="work", bufs=4) as wpool,
            tc.tile_pool(name="oh", bufs=6) as opool,
            tc.tile_pool(name="pt", bufs=2, space="PSUM") as pt_pool,
            tc.tile_pool(name="ph", bufs=2, space="PSUM") as ph_pool,
            tc.tile_pool(name="pm", bufs=2, space="PSUM") as pm_pool,
            tc.tile_pool(name="pa", bufs=2, space="PSUM") as pa_pool,
        ):
            def cload(dram, shape, dtype=f32):
                t = cpool.tile(shape, dtype, name=dram.name + "_t")
                nc.sync.dma_start(out=t[:], in_=dram[:])
                return t

            iota512_t = cload(iota512_d, [P, 512], f32)
            ident_t = cload(ident_d, [P, P], bf)
            mw1_sd_t = cload(mw1_sd_d, [2 * DIN, 2 * DOUT], bf)
            mb1_t = cload(mb1_d, [2 * DOUT, 1])
            mw2_t = cload(mw2_d, [2 * DOUT, DOUT], bf)
            mb2row_t = cload(mb2row_d, [1, DOUT], bf)
            dwrelT_t = cload(dwrelT_d, [P, btot])
            invN_t = cload(invN_d, [P, nw])
            hasrow_t = cload(hasrow_d, [1, npad], bf)
            uw1_t = cload(uw1_d, [DIN + DOUT, DOUT], bf)
            ub1_t = cload(ub1_d, [DOUT, 1])
            uw2_t = cload(uw2_d, [DOUT, DOUT], bf)
            ub2_t = cload(ub2_d, [DOUT, 1])
            lng_t = cload(lng_d, [P, DOUT])
            lnb_t = cload(lnb_d, [P, DOUT])

            eps_t = cpool.tile([P, 1], f32, name="eps_t")
            nc.vector.memset(eps_t[:], 1e-5)

            # per-4-window update-MLP input chunks:
            # rows 0:64 = aggT, rows 64:128 = xT
            UT = 512
            nchunk = (npad + UT - 1) // UT
            upd_c = []
            for k in range(nchunk):
                cw = min(UT, npad - k * UT)
                t = cpool.tile([P, cw], bf, name=f"upd_c{k}")
                nc.sync.dma_start(out=t[DOUT:P, :],
                                  in_=xTloc_d[:, k * UT:k * UT + cw])
                upd_c.append(t)

            # LN intermediates parked per chunk; sqrt batched at the end
            zc_all = [cpool.tile([P, 4 * DOUT], f32, name=f"zc_all{k}")
                      for k in range(nchunk)]
            red2_all = cpool.tile([P, 4 * nchunk], f32, name="red2_all")

            pa_cur = {}
            upd_done = [False] * nchunk

            def emit_upd(k):
                # ---- update MLP + LayerNorm for node chunk k ----
                upd_done[k] = True
                u0 = k * UT
                cw = min(UT, npad - u0)
                nj = cw // P
                upd_in = upd_c[k]
                pu = ph_pool.tile([P, 512], f32, tag="ph", name=f"pu_{u0}")
                nc.tensor.matmul(pu[0:DOUT, 0:cw], uw1_t[:],
                                 upd_in[:, 0:cw], start=True, stop=True)
                uh_sb = wpool.tile([DOUT, UT], bf, tag="uh", name=f"uh_{u0}")
                nc.scalar.activation(out=uh_sb[:, 0:cw],
                                     in_=pu[0:DOUT, 0:cw],
                                     func=ACT.Silu, bias=ub1_t[:, 0:1])
                pz = pt_pool.tile([P, 512], f32, tag="pt", name=f"pz_{u0}")
                nc.tensor.matmul(pz[0:DOUT, 0:cw], uw2_t[:], uh_sb[:, 0:cw],
                                 start=True, stop=True)
                zT_sb = wpool.tile([DOUT, UT], bf, tag="zT", name=f"zT_{u0}")
                nc.scalar.activation(out=zT_sb[:, 0:cw], in_=pz[0:DOUT, 0:cw],
                                     func=ACT.Identity, bias=ub2_t[:, 0:1])

                pz2 = pm_pool.tile([P, 4 * DOUT], bf, tag="pm",
                                   name=f"pz2_{u0}")
                for j in range(nj):
                    nc.tensor.transpose(
                        out=pz2[:, j * DOUT:(j + 1) * DOUT],
                        in_=zT_sb[:, j * P:(j + 1) * P],
                        identity=ident_t[0:DOUT, 0:DOUT])
                # LN phase A: mean-center + variance sum; sqrt deferred
                zc = zc_all[k]
                red = wpool.tile([P, 4], f32, tag="red", name=f"red_{u0}")
                z3 = pz2[:, 0:nj * DOUT].rearrange("p (j d) -> p j d", d=DOUT)
                nc.vector.tensor_reduce(out=red[:, 0:nj], in_=z3, axis=AX.X,
                                        op=OP.add)
                nc.vector.tensor_scalar_mul(red[:, 0:nj], red[:, 0:nj],
                                            -1.0 / DOUT)
                zc3 = zc[:, 0:nj * DOUT].rearrange("p (j d) -> p j d", d=DOUT)
                nc.vector.tensor_tensor(
                    out=zc3, in0=z3,
                    in1=red[:, 0:nj, None].to_broadcast([P, nj, DOUT]),
                    op=OP.add)
                sq = wpool.tile([P, 4 * DOUT], f32, tag="sq", name=f"sq_{u0}")
                sq3 = sq[:, 0:nj * DOUT].rearrange("p (j d) -> p j d", d=DOUT)
                nc.vector.tensor_tensor(out=sq3, in0=zc3, in1=zc3, op=OP.mult)
                nc.vector.tensor_reduce(out=red2_all[:, 4 * k:4 * k + nj],
                                        in_=sq3, axis=AX.X, op=OP.add)

            def emit_ln_final():
                # batched sqrt + reciprocal, then scale/affine/store per chunk
                sd = cpool.tile([P, 4 * nchunk], f32, name="sd_all")
                nc.scalar.activation(out=sd[:], in_=red2_all[:],
                                     func=ACT.Sqrt, scale=1.0 / DOUT,
                                     bias=eps_t[:, 0:1])
                rs = cpool.tile([P, 4 * nchunk], f32, name="rs_all")
                nc.vector.reciprocal(out=rs[:], in_=sd[:])
                for k in range(nchunk):
                    u0 = k * UT
                    cw = min(UT, npad - u0)
                    nj = cw // P
                    zc = zc_all[k]
                    zc3 = zc[:, 0:nj * DOUT].rearrange("p (j d) -> p j d",
                                                       d=DOUT)
                    zn = wpool.tile([P, 4 * DOUT], f32, tag="zn",
                                    name=f"zn_{u0}")
                    zn3 = zn[:, 0:nj * DOUT].rearrange("p (j d) -> p j d",
                                                       d=DOUT)
                    nc.vector.tensor_tensor(
                        out=zn3, in0=zc3,
                        in1=rs[:, 4 * k:4 * k + nj, None]
                            .to_broadcast([P, nj, DOUT]),
                        op=OP.mult)
                    for j in range(nj):
                        js = slice(j * DOUT, (j + 1) * DOUT)
                        nc.vector.tensor_tensor(out=zn[:, js], in0=zn[:, js],
                                                in1=lng_t[:], op=OP.mult)
                        nc.vector.tensor_tensor(out=zn[:, js], in0=zn[:, js],
                                                in1=lnb_t[:], op=OP.add)
                    od = out_d[u0:u0 + cw, :].rearrange(
                        "(j p) d -> p j d", p=P)
                    zn3o = zn[:, 0:nj * DOUT].rearrange(
                        "p (j d) -> p j d", d=DOUT)
                    nc.sync.dma_start(out=od, in_=zn3o)

            iota3 = iota512_t[:].rearrange("p (j c) -> p j c", c=P)

            for g0 in range(0, btot, 4):
                xp = gpool.tile([P, 512], bf, tag="xp", name=f"xp_{g0}")
                nc.sync.dma_start(out=xp[:],
                                  in_=vT_d[:, g0 * P:(g0 + 4) * P])

                oh4 = opool.tile([P, 4, P], bf, tag="oh", name=f"oh_{g0}")
                nc.vector.tensor_tensor(
                    out=oh4[:],
                    in0=iota3,
                    in1=dwrelT_t[:, g0:g0 + 4, None].to_broadcast([P, 4, P]),
                    op=OP.is_equal)

                ph = ph_pool.tile([P, 512], f32, tag="ph", name=f"ph_{g0}")
                nc.tensor.matmul(ph[:], mw1_sd_t[:], xp[:],
                                 start=True, stop=True)
                hT_sb = wpool.tile([P, 512], bf, tag="hT", name=f"hT_{g0}")
                nc.scalar.activation(out=hT_sb[:], in_=ph[:],
                                     func=ACT.Silu, bias=mb1_t[:, 0:1])

                pm = pm_pool.tile([P, 4 * DOUT], f32, tag="pm",
                                  name=f"pm_{g0}")
                for j in range(4):
                    nc.tensor.matmul(pm[:, j * DOUT:(j + 1) * DOUT],
                                     hT_sb[:, j * P:(j + 1) * P],
                                     mw2_t[:], start=True, stop=True)
                msg_sb = wpool.tile([P, 4 * DOUT], bf, tag="msg",
                                    name=f"msg_{g0}")
                nc.scalar.copy(out=msg_sb[:], in_=pm[:])

                for j in range(4):
                    g = g0 + j
                    w = block_window[g]
                    if g == wfirst[w]:
                        pa_cur[w] = pa_pool.tile([P, DOUT], f32, tag="pa",
                                                 name=f"pa_w{w}")
                    nc.tensor.matmul(
                        pa_cur[w][:],
                        oh4[:, j, :],
                        msg_sb[:, j * DOUT:(j + 1) * DOUT],
                        start=(g == wfirst[w]), stop=(g == wlast[w]),
                        skip_group_check=True)
                    if g != wlast[w]:
                        continue
                    # ---- window flush ----
                    s_nT = wpool.tile([P, DOUT], bf, tag="snt",
                                      name=f"snt_{w}")
                    nc.vector.tensor_tensor(
                        out=s_nT[:], in0=pa_cur[w][:],
                        in1=invN_t[:, w:w + 1].to_broadcast([P, DOUT]),
                        op=OP.mult)
                    del pa_cur[w]
                    agg_ps = pt_pool.tile([P, P], f32, tag="pt",
                                          name=f"agg_{w}")
                    nc.tensor.matmul(agg_ps[0:DOUT, :], s_nT[:], ident_t[:],
                                     start=True, stop=False,
                                     skip_group_check=True)
                    wc = slice(w * P, (w + 1) * P)
                    nc.tensor.matmul(agg_ps[0:DOUT, :], mb2row_t[:],
                                     hasrow_t[:, wc], start=False, stop=True,
                                     skip_group_check=True)
                    kc = w // 4
                    uc = slice((w % 4) * P, (w % 4 + 1) * P)
                    nc.scalar.copy(out=upd_c[kc][0:DOUT, uc],
                                   in_=agg_ps[0:DOUT, :])
                    if w == min(4 * (w // 4) + 4, nw) - 1:
                        emit_upd(w // 4)

            for k in range(nchunk):
                if not upd_done[k]:
                    emit_upd(k)
            emit_ln_final()

    nc.compile()
    return nc


# ---------------------------------------------------------------------------
# Entry point
# ---------------------------------------------------------------------------

last_results = None


def kernel(x, edge_index, edge_vec, edge_len,
           mw1, mb1, mw2, mb2, uw1, ub1, uw2, ub2, ln_g, ln_b):
    global last_results
    import os
    from concourse.bass_utils import run_bass_kernel_spmd

    struct, in_maps = _build_host_data(
        x, edge_index, edge_len, mw1, mb1, mw2, mb2,
        uw1, ub1, uw2, ub2, ln_g, ln_b)

    key = (struct["n"], struct["btot"], struct["bws"])
    if key not in _prog_cache:
        _prog_cache[key] = _build_program(struct)
    nc = _prog_cache[key]

    kw = {}
    if os.environ.get("K_TRACE", ""):
        try:
            import profile_shim
            profile_shim.install()
        except ImportError:
            pass
        kw = dict(trace=True, trace_cores=list(range(NCORES)),
                  tmpdir="/tmp/ntff_out")
    res = run_bass_kernel_spmd(nc, in_maps, core_ids=list(range(NCORES)), **kw)
    last_results = res
    nloc = struct["nloc"]
    out = np.concatenate([res.results[c]["out"][:nloc] for c in range(NCORES)],
                         axis=0)
    return out.astype(np.float32)


# revision 16
# speedup vs baseline: 2.8157x; 1.2125x over previous
"""GNN message-passing layer (EquivariantMPLayer) on 8 Trainium2 NeuronCores.

Sharding: edges are sharded by destination-node range (dst // (N/8)) so each
core aggregates its own node range locally -- no collectives needed.

Host prep does the gather: for each core's dst-sorted edge list, the host
builds a feature-major bf16 stream vT[128, epad] where each edge column is
v = [x[src]; x[dst]] + M @ rbf, with M = (mw1_sd^T)^{-1} @ mw1_r^T. Since
mw1_sd is square and invertible, mw1_sd^T @ v == mw1_sd^T @ [xs;xd] +
mw1_r^T @ rbf exactly, so the RBF term rides along in the same 128-row
matmul and the device does no gathers, no transposes and no rbf matmul.

Device pipeline per 4-block supertile (512 edges):
  - one sequential DMA of vT columns (128 KB)
  - one DVE op builds 4 one-hot scatter blocks: oh[e, n] = (iota == dwrel)
  - L1 matmul (mw1_sd stationary, vT moving) -> ph[128 hd, 512] PSUM
  - Silu (ACT, fused mb1 bias) -> hT bf16
  - L2 per block: lhsT=hT block -> msg edge-major [128 e, 64] PSUM -> bf16
  - scatter per block: lhsT=oh, rhs=msg -> S[node, dout] PSUM accumulated
    over the window's blocks
  - window flush: DVE inv-scale (per-node 1/max(cnt,1)), PE transpose to
    [dout, node], += mb2 (x) hasrow via K=1 matmul, copy into update chunk
Then an update MLP + LayerNorm over the core's nodes, written row-major.
"""

import numpy as np

N = 50000
E = 800000
DIN = 64
DOUT = 64
NB = 16
MAX_RADIUS = 10.0
NCORES = 8
P = 128

_prog_cache = {}


# ---------------------------------------------------------------------------
# Host-side structure / metadata
# ---------------------------------------------------------------------------

def _build_host_data(x, edge_index, edge_len, mw1, mb1, mw2, mb2,
                     uw1, ub1, uw2, ub2, ln_g, ln_b,
                     n=N, ncores=NCORES):
    import ml_dtypes
    bf16 = ml_dtypes.bfloat16

    nloc = n // ncores
    nw = (nloc + P - 1) // P
    npad = nw * P

    src = np.asarray(edge_index[0], dtype=np.int64)
    dst = np.asarray(edge_index[1], dtype=np.int64)
    x = np.asarray(x, dtype=np.float32)
    el = np.asarray(edge_len, dtype=np.float32)[:, 0]

    centers = np.linspace(0.0, MAX_RADIUS, NB, dtype=np.float64)
    width = (centers[1] - centers[0]) * 0.5
    rbf_all = np.exp(-((el[:, None].astype(np.float64) - centers) ** 2)
                     / (2.0 * width ** 2)).astype(np.float32)  # [E, 16]

    # fold mw1_r into the shipped edge vectors:
    # v = [xs; xd] + M @ rbf with M = (mw1_sd_bf^T)^-1 @ mw1_r^T (f64 solve
    # against the bf16-rounded mw1_sd actually used on device)
    mw1 = np.asarray(mw1, np.float32)
    mw1_sd_bf = mw1[:2 * DIN].astype(np.float16)
    mw1_r = mw1[2 * DIN:]
    M = np.linalg.solve(mw1_sd_bf.astype(np.float64).T,
                        mw1_r.astype(np.float64).T)  # [128, 16]
    Mt = M.T.astype(np.float32)  # [16, 128]

    core_of = dst // nloc
    per_core = []
    cnt_cw = np.zeros((ncores, nw), dtype=np.int64)
    for c in range(ncores):
        eids = np.nonzero(core_of == c)[0]
        dloc = (dst[eids] - c * nloc).astype(np.int64)
        order = np.argsort(dloc, kind="stable")
        eids = eids[order]
        dloc = dloc[order]
        w_of = dloc // P
        cnt_cw[c] = np.bincount(w_of, minlength=nw)
        per_core.append((eids, dloc, w_of))

    # per-window block counts, equalized across cores; pad total to %16
    # (16 blocks = one 4-supertile DMA chunk of vT)
    bws = np.maximum(1, (cnt_cw.max(axis=0) + P - 1) // P)  # [nw]
    bws[-1] += (-int(bws.sum())) % 16
    btot = int(bws.sum())
    epad = btot * P

    block_window = []
    for w in range(nw):
        block_window += [w] * int(bws[w])
    block_window = np.array(block_window)
    boff = np.concatenate([[0], np.cumsum(bws)])  # block offset per window

    in_maps = []
    for c in range(ncores):
        eids, dloc, w_of = per_core[c]
        ne = len(eids)
        # position of each edge inside its window's block range
        # edges are dst-sorted so within a window they are consecutive
        wstart = np.concatenate([[0], np.cumsum(cnt_cw[c])])
        pos_in_w = np.arange(ne) - wstart[w_of]
        slot = boff[w_of] * P + pos_in_w  # global padded slot per edge

        vpair = np.zeros((epad, 2 * DIN), dtype=np.float32)
        vpair[slot, :DIN] = x[src[eids]]
        vpair[slot, DIN:] = x[dst[eids]]
        vpair[slot] += rbf_all[eids] @ Mt
        # supertile-contiguous layout: [nchk, 128, 2048] so each 4-supertile
        # DMA reads one contiguous 512 KB block
        vT = np.ascontiguousarray(vpair.T).astype(np.float16)  # [128, epad]
        nchk = epad // 2048
        v4 = np.ascontiguousarray(
            vT.reshape(P, nchk, 2048).transpose(1, 0, 2)
        ).reshape(nchk * P, 2048)

        dwrelT = np.full((P, btot), 999.0, dtype=np.float32)
        dwrelT[pos_in_w % P, boff[w_of] + pos_in_w // P] = \
            (dloc - w_of * P).astype(np.float32)

        cnt_n = np.zeros(npad, dtype=np.float32)
        cnt_n[:nloc] = np.bincount(dloc, minlength=nloc).astype(np.float32)
        invN = np.ascontiguousarray(
            (1.0 / np.maximum(cnt_n, 1.0)).reshape(nw, P).T)  # [128, nw]
        has = (cnt_n > 0).astype(np.float32)

        xt_loc = np.zeros((DIN, npad), dtype=bf16)
        xt_loc[:, :nloc] = x[c * nloc:(c + 1) * nloc].T.astype(bf16)

        iota512 = np.broadcast_to(
            (np.arange(512) % P).astype(np.float32)[None, :], (P, 512)).copy()

        m = {
            "vT": v4,
            "dwrelT": dwrelT,
            "invN": invN,
            "xTloc": xt_loc,
            "hasrow": has.reshape(1, npad).astype(bf16),
            "mb2row": np.asarray(mb2, np.float32).reshape(1, DOUT)
                        .astype(bf16),
            "mw1_sd": mw1_sd_bf,
            "mb1": np.asarray(mb1, np.float32).reshape(2 * DOUT, 1).copy(),
            "mw2": np.asarray(mw2, np.float32).astype(bf16),
            # upd layout is [agg; x] -> swap uw1 row blocks to match
            "uw1": np.concatenate([np.asarray(uw1, np.float32)[DIN:],
                                   np.asarray(uw1, np.float32)[:DIN]],
                                  axis=0).astype(bf16),
            "ub1": np.asarray(ub1, np.float32).reshape(DOUT, 1).copy(),
            "uw2": np.asarray(uw2, np.float32).astype(bf16),
            "ub2": np.asarray(ub2, np.float32).reshape(DOUT, 1).copy(),
            "lng": np.broadcast_to(np.asarray(ln_g, np.float32)[None, :],
                                   (P, DOUT)).copy(),
            "lnb": np.broadcast_to(np.asarray(ln_b, np.float32)[None, :],
                                   (P, DOUT)).copy(),
            "iota512": iota512,
            "ident": np.eye(P, dtype=np.float32).astype(bf16),
        }
        in_maps.append(m)

    struct = dict(n=n, nloc=nloc, nw=nw, npad=npad, btot=btot, epad=epad,
                  bws=tuple(int(v) for v in bws),
                  block_window=tuple(int(v) for v in block_window))
    return struct, in_maps


# ---------------------------------------------------------------------------
# Device program
# ---------------------------------------------------------------------------

def _build_program(struct):
    import concourse.bass as bass
    import concourse.mybir as mybir
    import concourse.tile as tile
    from concourse import bacc

    f32 = mybir.dt.float32
    bf = mybir.dt.bfloat16
    f16 = mybir.dt.float16
    n, nloc, nw, npad = (struct["n"], struct["nloc"], struct["nw"],
                         struct["npad"])
    btot, epad = struct["btot"], struct["epad"]
    block_window = struct["block_window"]

    wfirst = {}
    wlast = {}
    for g, w in enumerate(block_window):
        wfirst.setdefault(w, g)
        wlast[w] = g

    nc = bacc.Bacc("TRN2", target_bir_lowering=False, debug=False,
                   enable_asserts=False, num_devices=NCORES)

    vT_d = nc.dram_tensor("vT", [(btot // 16) * P, 2048], f16,
                          kind="ExternalInput")
    dwrelT_d = nc.dram_tensor("dwrelT", [P, btot], f32, kind="ExternalInput")
    invN_d = nc.dram_tensor("invN", [P, nw], f32, kind="ExternalInput")
    xTloc_d = nc.dram_tensor("xTloc", [DIN, npad], bf, kind="ExternalInput")
    hasrow_d = nc.dram_tensor("hasrow", [1, npad], bf, kind="ExternalInput")
    mb2row_d = nc.dram_tensor("mb2row", [1, DOUT], bf, kind="ExternalInput")
    mw1_sd_d = nc.dram_tensor("mw1_sd", [2 * DIN, 2 * DOUT], f16,
                              kind="ExternalInput")
    mb1_d = nc.dram_tensor("mb1", [2 * DOUT, 1], f32, kind="ExternalInput")
    mw2_d = nc.dram_tensor("mw2", [2 * DOUT, DOUT], bf, kind="ExternalInput")
    uw1_d = nc.dram_tensor("uw1", [DIN + DOUT, DOUT], bf,
                           kind="ExternalInput")
    ub1_d = nc.dram_tensor("ub1", [DOUT, 1], f32, kind="ExternalInput")
    uw2_d = nc.dram_tensor("uw2", [DOUT, DOUT], bf, kind="ExternalInput")
    ub2_d = nc.dram_tensor("ub2", [DOUT, 1], f32, kind="ExternalInput")
    lng_d = nc.dram_tensor("lng", [P, DOUT], f32, kind="ExternalInput")
    lnb_d = nc.dram_tensor("lnb", [P, DOUT], f32, kind="ExternalInput")
    iota512_d = nc.dram_tensor("iota512", [P, 512], f32,
                               kind="ExternalInput")
    ident_d = nc.dram_tensor("ident", [P, P], bf, kind="ExternalInput")
    out_d = nc.dram_tensor("out", [npad, DOUT], f32, kind="ExternalOutput")

    AX = mybir.AxisListType
    OP = mybir.AluOpType
    ACT = mybir.ActivationFunctionType

    with tile.TileContext(nc) as tc:
        with (
            tc.tile_pool(name="const", bufs=1) as cpool,
            tc.tile_pool(name="gath", bufs=4) as gpool,
            tc.tile_pool(name="work", bufs=4) as wpool,
            tc.tile_pool(name="oh", bufs=6) as opool,
            tc.tile_pool(name="pt", bufs=2, space="PSUM") as pt_pool,
            tc.tile_pool(name="ph", bufs=2, space="PSUM") as ph_pool,
            tc.tile_pool(name="pm", bufs=2, space="PSUM") as pm_pool,
            tc.tile_pool(name="pa", bufs=2, space="PSUM") as pa_pool,
        ):
            def cload(dram, shape, dtype=f32):
                t = cpool.tile(shape, dtype, name=dram.name + "_t")
                nc.sync.dma_start(out=t[:], in_=dram[:])
                return t

            iota512_t = cload(iota512_d, [P, 512], f32)
            ident_t = cload(ident_d, [P, P], bf)
            mw1_sd_t = cload(mw1_sd_d, [2 * DIN, 2 * DOUT], f16)
            mb1_t = cload(mb1_d, [2 * DOUT, 1])
            mw2_t = cload(mw2_d, [2 * DOUT, DOUT], bf)
            mb2row_t = cload(mb2row_d, [1, DOUT], bf)
            dwrelT_t = cload(dwrelT_d, [P, btot])
            invN_t = cload(invN_d, [P, nw])
            hasrow_t = cload(hasrow_d, [1, npad], bf)
            uw1_t = cload(uw1_d, [DIN + DOUT, DOUT], bf)
            ub1_t = cload(ub1_d, [DOUT, 1])
            uw2_t = cload(uw2_d, [DOUT, DOUT], bf)
            ub2_t = cload(ub2_d, [DOUT, 1])
            lng_t = cload(lng_d, [P, DOUT])
            lnb_t = cload(lnb_d, [P, DOUT])

            eps_t = cpool.tile([P, 1], f32, name="eps_t")
            nc.vector.memset(eps_t[:], 1e-5)

            # per-4-window update-MLP input chunks:
            # rows 0:64 = aggT, rows 64:128 = xT
            UT = 512
            nchunk = (npad + UT - 1) // UT
            upd_c = []
            for k in range(nchunk):
                cw = min(UT, npad - k * UT)
                t = cpool.tile([P, cw], bf, name=f"upd_c{k}")
                nc.sync.dma_start(out=t[DOUT:P, :],
                                  in_=xTloc_d[:, k * UT:k * UT + cw])
                upd_c.append(t)

            # LN intermediates parked per chunk; sqrt batched at the end
            zc_all = [cpool.tile([P, 4 * DOUT], f32, name=f"zc_all{k}")
                      for k in range(nchunk)]
            red2_all = cpool.tile([P, 4 * nchunk], f32, name="red2_all")

            pa_cur = {}
            upd_done = [False] * nchunk

            def emit_upd(k):
                # ---- update MLP + LayerNorm for node chunk k ----
                upd_done[k] = True
                u0 = k * UT
                cw = min(UT, npad - u0)
                nj = cw // P
                upd_in = upd_c[k]
                pu = ph_pool.tile([P, 512], f32, tag="ph", name=f"pu_{u0}")
                nc.tensor.matmul(pu[0:DOUT, 0:cw], uw1_t[:],
                                 upd_in[:, 0:cw], start=True, stop=True)
                uh_sb = wpool.tile([DOUT, UT], bf, tag="uh", name=f"uh_{u0}")
                nc.scalar.activation(out=uh_sb[:, 0:cw],
                                     in_=pu[0:DOUT, 0:cw],
                                     func=ACT.Silu, bias=ub1_t[:, 0:1])
                pz = pt_pool.tile([P, 512], f32, tag="pt", name=f"pz_{u0}")
                nc.tensor.matmul(pz[0:DOUT, 0:cw], uw2_t[:], uh_sb[:, 0:cw],
                                 start=True, stop=True)
                zT_sb = wpool.tile([DOUT, UT], bf, tag="zT", name=f"zT_{u0}")
                nc.scalar.activation(out=zT_sb[:, 0:cw], in_=pz[0:DOUT, 0:cw],
                                     func=ACT.Identity, bias=ub2_t[:, 0:1])

                pz2 = pm_pool.tile([P, 4 * DOUT], bf, tag="pm",
                                   name=f"pz2_{u0}")
                for j in range(nj):
                    nc.tensor.transpose(
                        out=pz2[:, j * DOUT:(j + 1) * DOUT],
                        in_=zT_sb[:, j * P:(j + 1) * P],
                        identity=ident_t[0:DOUT, 0:DOUT])
                # LN phase A: mean-center + variance sum; sqrt deferred
                zc = zc_all[k]
                red = wpool.tile([P, 4], f32, tag="red", name=f"red_{u0}")
                z3 = pz2[:, 0:nj * DOUT].rearrange("p (j d) -> p j d", d=DOUT)
                nc.vector.tensor_reduce(out=red[:, 0:nj], in_=z3, axis=AX.X,
                                        op=OP.add)
                nc.vector.tensor_scalar_mul(red[:, 0:nj], red[:, 0:nj],
                                            -1.0 / DOUT)
                zc3 = zc[:, 0:nj * DOUT].rearrange("p (j d) -> p j d", d=DOUT)
                nc.vector.tensor_tensor(
                    out=zc3, in0=z3,
                    in1=red[:, 0:nj, None].to_broadcast([P, nj, DOUT]),
                    op=OP.add)
                sq = wpool.tile([P, 4 * DOUT], f32, tag="sq", name=f"sq_{u0}")
                sq3 = sq[:, 0:nj * DOUT].rearrange("p (j d) -> p j d", d=DOUT)
                nc.vector.tensor_tensor(out=sq3, in0=zc3, in1=zc3, op=OP.mult)
                nc.vector.tensor_reduce(out=red2_all[:, 4 * k:4 * k + nj],
                                        in_=sq3, axis=AX.X, op=OP.add)

            def emit_ln_final():
                # batched sqrt + reciprocal, then scale/affine/store per chunk
                sd = cpool.tile([P, 4 * nchunk], f32, name="sd_all")
                nc.scalar.activation(out=sd[:], in_=red2_all[:],
                                     func=ACT.Sqrt, scale=1.0 / DOUT,
                                     bias=eps_t[:, 0:1])
                rs = cpool.tile([P, 4 * nchunk], f32, name="rs_all")
                nc.vector.reciprocal(out=rs[:], in_=sd[:])
                for k in range(nchunk):
                    u0 = k * UT
                    cw = min(UT, npad - u0)
                    nj = cw // P
                    zc = zc_all[k]
                    zc3 = zc[:, 0:nj * DOUT].rearrange("p (j d) -> p j d",
                                                       d=DOUT)
                    zn = wpool.tile([P, 4 * DOUT], f32, tag="zn",
                                    name=f"zn_{u0}")
                    zn3 = zn[:, 0:nj * DOUT].rearrange("p (j d) -> p j d",
                                                       d=DOUT)
                    nc.vector.tensor_tensor(
                        out=zn3, in0=zc3,
                        in1=rs[:, 4 * k:4 * k + nj, None]
                            .to_broadcast([P, nj, DOUT]),
                        op=OP.mult)
                    nc.vector.scalar_tensor_tensor(
                        out=zn3, in0=zn3, scalar=1.0,
                        in1=lng_t[:, None, :].to_broadcast([P, nj, DOUT]),
                        op0=OP.mult, op1=OP.mult)
                    nc.vector.tensor_tensor(
                        out=zn3, in0=zn3,
                        in1=lnb_t[:, None, :].to_broadcast([P, nj, DOUT]),
                        op=OP.add)
                    od = out_d[u0:u0 + cw, :].rearrange(
                        "(j p) d -> p j d", p=P)
                    zn3o = zn[:, 0:nj * DOUT].rearrange(
                        "p (j d) -> p j d", d=DOUT)
                    nc.sync.dma_start(out=od, in_=zn3o)

            iota3 = iota512_t[:].rearrange("p (j c) -> p j c", c=P)

            for g0 in range(0, btot, 4):
                st = g0 // 4
                if st % 4 == 0:
                    xp4 = gpool.tile([P, 2048], f16, tag="xp",
                                     name=f"xp_{g0}")
                    nc.sync.dma_start(
                        out=xp4[:],
                        in_=vT_d[(st // 4) * P:(st // 4 + 1) * P, :])
                xp = xp4[:, (st % 4) * 512:(st % 4 + 1) * 512]

                oh4 = opool.tile([P, 4, P], bf, tag="oh", name=f"oh_{g0}")
                nc.vector.tensor_tensor(
                    out=oh4[:],
                    in0=iota3,
                    in1=dwrelT_t[:, g0:g0 + 4, None].to_broadcast([P, 4, P]),
                    op=OP.is_equal)

                ph = ph_pool.tile([P, 512], f32, tag="ph", name=f"ph_{g0}")
                nc.tensor.matmul(ph[:], mw1_sd_t[:], xp,
                                 start=True, stop=True)
                hT_sb = wpool.tile([P, 512], bf, tag="hT", name=f"hT_{g0}")
                nc.scalar.activation(out=hT_sb[:], in_=ph[:],
                                     func=ACT.Silu, bias=mb1_t[:, 0:1])

                pm = pm_pool.tile([P, 4 * DOUT], f32, tag="pm",
                                  name=f"pm_{g0}")
                for j in range(4):
                    nc.tensor.matmul(pm[:, j * DOUT:(j + 1) * DOUT],
                                     hT_sb[:, j * P:(j + 1) * P],
                                     mw2_t[:], start=True, stop=True)
                msg_sb = wpool.tile([P, 4 * DOUT], bf, tag="msg",
                                    name=f"msg_{g0}")
                nc.vector.tensor_copy(out=msg_sb[:], in_=pm[:])

                for j in range(4):
                    g = g0 + j
                    w = block_window[g]
                    if g == wfirst[w]:
                        pa_cur[w] = pa_pool.tile([P, DOUT], f32, tag="pa",
                                                 name=f"pa_w{w}")
                    nc.tensor.matmul(
                        pa_cur[w][:],
                        oh4[:, j, :],
                        msg_sb[:, j * DOUT:(j + 1) * DOUT],
                        start=(g == wfirst[w]), stop=(g == wlast[w]),
                        skip_group_check=True)
                    if g != wlast[w]:
                        continue
                    # ---- window flush ----
                    s_nT = wpool.tile([P, DOUT], bf, tag="snt",
                                      name=f"snt_{w}")
                    nc.vector.tensor_tensor(
                        out=s_nT[:], in0=pa_cur[w][:],
                        in1=invN_t[:, w:w + 1].to_broadcast([P, DOUT]),
                        op=OP.mult)
                    del pa_cur[w]
                    agg_ps = pt_pool.tile([P, P], f32, tag="pt",
                                          name=f"agg_{w}")
                    nc.tensor.matmul(agg_ps[0:DOUT, :], s_nT[:], ident_t[:],
                                     start=True, stop=False,
                                     skip_group_check=True)
                    wc = slice(w * P, (w + 1) * P)
                    nc.tensor.matmul(agg_ps[0:DOUT, :], mb2row_t[:],
                                     hasrow_t[:, wc], start=False, stop=True,
                                     skip_group_check=True)
                    kc = w // 4
                    uc = slice((w % 4) * P, (w % 4 + 1) * P)
                    nc.scalar.copy(out=upd_c[kc][0:DOUT, uc],
                                   in_=agg_ps[0:DOUT, :])
                    if w == min(4 * (w // 4) + 4, nw) - 1:
                        emit_upd(w // 4)

            for k in range(nchunk):
                if not upd_done[k]:
                    emit_upd(k)
            emit_ln_final()

    nc.compile()
    return nc


# ---------------------------------------------------------------------------
# Entry point
# ---------------------------------------------------------------------------

last_results = None


def kernel(x, edge_index, edge_vec, edge_len,
           mw1, mb1, mw2, mb2, uw1, ub1, uw2, ub2, ln_g, ln_b):
    global last_results
    import os
    from concourse.bass_utils import run_bass_kernel_spmd

    struct, in_maps = _build_host_data(
        x, edge_index, edge_len, mw1, mb1, mw2, mb2,
        uw1, ub1, uw2, ub2, ln_g, ln_b)

    key = (struct["n"], struct["btot"], struct["bws"])
    if key not in _prog_cache:
        _prog_cache[key] = _build_program(struct)
    nc = _prog_cache[key]

    kw = {}
    if os.environ.get("K_TRACE", ""):
        try:
            import profile_shim
            profile_shim.install()
        except ImportError:
            pass
        kw = dict(trace=True, trace_cores=list(range(NCORES)),
                  tmpdir="/tmp/ntff_out")
    res = run_bass_kernel_spmd(nc, in_maps, core_ids=list(range(NCORES)), **kw)
    last_results = res
    nloc = struct["nloc"]
    out = np.concatenate([res.results[c]["out"][:nloc] for c in range(NCORES)],
                         axis=0)
    return out.astype(np.float32)


# revision 26
# speedup vs baseline: 2.9221x; 1.0378x over previous
"""GNN message-passing layer (EquivariantMPLayer) on 8 Trainium2 NeuronCores.

Sharding: edges are sharded by destination-node range (dst // (N/8)) so each
core aggregates its own node range locally -- no collectives needed.

Host prep does the gather: for each core's dst-sorted edge list, the host
builds a feature-major bf16 stream vT[128, epad] where each edge column is
v = [x[src]; x[dst]] + M @ rbf, with M = (mw1_sd^T)^{-1} @ mw1_r^T. Since
mw1_sd is square and invertible, mw1_sd^T @ v == mw1_sd^T @ [xs;xd] +
mw1_r^T @ rbf exactly, so the RBF term rides along in the same 128-row
matmul and the device does no gathers, no transposes and no rbf matmul.

Device pipeline per 4-block supertile (512 edges):
  - one sequential DMA of vT columns (128 KB)
  - one DVE op builds 4 one-hot scatter blocks: oh[e, n] = (iota == dwrel)
  - L1 matmul (mw1_sd stationary, vT moving) -> ph[128 hd, 512] PSUM
  - Silu (ACT, fused mb1 bias) -> hT bf16
  - L2 per block: lhsT=hT block -> msg edge-major [128 e, 64] PSUM -> bf16
  - scatter per block: lhsT=oh, rhs=msg -> S[node, dout] PSUM accumulated
    over the window's blocks
  - window flush: DVE inv-scale (per-node 1/max(cnt,1)), PE transpose to
    [dout, node], += mb2 (x) hasrow via K=1 matmul, copy into update chunk
Then an update MLP + LayerNorm over the core's nodes, written row-major.
"""

import numpy as np

N = 50000
E = 800000
DIN = 64
DOUT = 64
NB = 16
MAX_RADIUS = 10.0
NCORES = 8
P = 128

_prog_cache = {}


# ---------------------------------------------------------------------------
# Host-side structure / metadata
# ---------------------------------------------------------------------------

def _build_host_data(x, edge_index, edge_len, mw1, mb1, mw2, mb2,
                     uw1, ub1, uw2, ub2, ln_g, ln_b,
                     n=N, ncores=NCORES):
    import ml_dtypes
    bf16 = ml_dtypes.bfloat16

    nloc = n // ncores
    nw = (nloc + P - 1) // P
    npad = nw * P

    src = np.asarray(edge_index[0], dtype=np.int64)
    dst = np.asarray(edge_index[1], dtype=np.int64)
    x = np.asarray(x, dtype=np.float32)
    el = np.asarray(edge_len, dtype=np.float32)[:, 0]

    centers = np.linspace(0.0, MAX_RADIUS, NB, dtype=np.float64)
    width = (centers[1] - centers[0]) * 0.5
    rbf_all = np.exp(-((el[:, None].astype(np.float64) - centers) ** 2)
                     / (2.0 * width ** 2)).astype(np.float32)  # [E, 16]

    # fold mw1_r into the shipped edge vectors:
    # v = [xs; xd] + M @ rbf with M = (mw1_sd_bf^T)^-1 @ mw1_r^T (f64 solve
    # against the bf16-rounded mw1_sd actually used on device)
    mw1 = np.asarray(mw1, np.float32)
    mw1_sd_bf = mw1[:2 * DIN].astype(np.float16)
    mw1_r = mw1[2 * DIN:]
    M = np.linalg.solve(mw1_sd_bf.astype(np.float64).T,
                        mw1_r.astype(np.float64).T)  # [128, 16]
    Mt = M.T.astype(np.float32)  # [16, 128]

    core_of = dst // nloc
    per_core = []
    cnt_cw = np.zeros((ncores, nw), dtype=np.int64)
    for c in range(ncores):
        eids = np.nonzero(core_of == c)[0]
        dloc = (dst[eids] - c * nloc).astype(np.int64)
        order = np.argsort(dloc, kind="stable")
        eids = eids[order]
        dloc = dloc[order]
        w_of = dloc // P
        cnt_cw[c] = np.bincount(w_of, minlength=nw)
        per_core.append((eids, dloc, w_of))

    # per-window block counts, equalized across cores; pad total to %16
    # (16 blocks = one 4-supertile DMA chunk of vT)
    bws = np.maximum(1, (cnt_cw.max(axis=0) + P - 1) // P)  # [nw]
    bws[-1] += (-int(bws.sum())) % 16
    btot = int(bws.sum())
    epad = btot * P

    block_window = []
    for w in range(nw):
        block_window += [w] * int(bws[w])
    block_window = np.array(block_window)
    boff = np.concatenate([[0], np.cumsum(bws)])  # block offset per window

    in_maps = []
    for c in range(ncores):
        eids, dloc, w_of = per_core[c]
        ne = len(eids)
        # position of each edge inside its window's block range
        # edges are dst-sorted so within a window they are consecutive
        wstart = np.concatenate([[0], np.cumsum(cnt_cw[c])])
        pos_in_w = np.arange(ne) - wstart[w_of]
        slot = boff[w_of] * P + pos_in_w  # global padded slot per edge

        vpair = np.zeros((epad, 2 * DIN), dtype=np.float32)
        vpair[slot, :DIN] = x[src[eids]]
        vpair[slot, DIN:] = x[dst[eids]]
        vpair[slot] += rbf_all[eids] @ Mt
        # supertile-contiguous layout: [nchk, 128, 2048] so each 4-supertile
        # DMA reads one contiguous 512 KB block
        vT = np.ascontiguousarray(vpair.T).astype(np.float16)  # [128, epad]
        nchk = epad // 2048
        v4 = np.ascontiguousarray(
            vT.reshape(P, nchk, 2048).transpose(1, 0, 2)
        ).reshape(nchk * P, 2048)

        dwrelT = np.full((P, btot), 999.0, dtype=np.float32)
        dwrelT[pos_in_w % P, boff[w_of] + pos_in_w // P] = \
            (dloc - w_of * P).astype(np.float32)
        dwrelT = dwrelT.astype(bf16)

        cnt_n = np.zeros(npad, dtype=np.float32)
        cnt_n[:nloc] = np.bincount(dloc, minlength=nloc).astype(np.float32)
        invN = np.ascontiguousarray(
            (1.0 / np.maximum(cnt_n, 1.0)).reshape(nw, P).T)  # [128, nw]
        has = (cnt_n > 0).astype(np.float32)

        xt_loc = np.zeros((DIN, npad), dtype=bf16)
        xt_loc[:, :nloc] = x[c * nloc:(c + 1) * nloc].T.astype(bf16)

        iota512 = np.broadcast_to(
            (np.arange(512) % P).astype(bf16)[None, :], (P, 512)).copy()

        m = {
            "vT": v4,
            "dwrelT": dwrelT,
            "invN": invN,
            "xTloc": xt_loc,
            "hasrow": has.reshape(1, npad).astype(bf16),
            # uw1_agg^T @ mb2: the update-MLP image of the mb2(x)hasrow
            # term, applied once per chunk instead of per window
            "bex": (np.asarray(uw1, np.float32)[DIN:].T
                    @ np.asarray(mb2, np.float32)).reshape(1, DOUT)
                   .astype(bf16),
            "mw1_sd": mw1_sd_bf,
            "mb1": np.asarray(mb1, np.float32).reshape(2 * DOUT, 1).copy(),
            "mw2": np.asarray(mw2, np.float32).astype(bf16),
            # upd layout is [agg; x] -> swap uw1 row blocks to match
            "uw1": np.concatenate([np.asarray(uw1, np.float32)[DIN:],
                                   np.asarray(uw1, np.float32)[:DIN]],
                                  axis=0).astype(bf16),
            "ub1": np.asarray(ub1, np.float32).reshape(DOUT, 1).copy(),
            "uw2": np.asarray(uw2, np.float32).astype(bf16),
            "ub2": np.asarray(ub2, np.float32).reshape(DOUT, 1).copy(),
            "lng": np.broadcast_to(np.asarray(ln_g, np.float32)[None, :],
                                   (P, DOUT)).copy(),
            "lnb": np.broadcast_to(np.asarray(ln_b, np.float32)[None, :],
                                   (P, DOUT)).copy(),
            "iota512": iota512,
            "ident": np.eye(P, dtype=np.float32).astype(bf16),
        }
        in_maps.append(m)

    struct = dict(n=n, nloc=nloc, nw=nw, npad=npad, btot=btot, epad=epad,
                  bws=tuple(int(v) for v in bws),
                  block_window=tuple(int(v) for v in block_window))
    return struct, in_maps


# ---------------------------------------------------------------------------
# Device program
# ---------------------------------------------------------------------------

def _build_program(struct):
    import concourse.bass as bass
    import concourse.mybir as mybir
    import concourse.tile as tile
    from concourse import bacc

    f32 = mybir.dt.float32
    bf = mybir.dt.bfloat16
    f16 = mybir.dt.float16
    n, nloc, nw, npad = (struct["n"], struct["nloc"], struct["nw"],
                         struct["npad"])
    btot, epad = struct["btot"], struct["epad"]
    block_window = struct["block_window"]

    wfirst = {}
    wlast = {}
    for g, w in enumerate(block_window):
        wfirst.setdefault(w, g)
        wlast[w] = g

    nc = bacc.Bacc("TRN2", target_bir_lowering=False, debug=False,
                   enable_asserts=False, num_devices=NCORES)

    vT_d = nc.dram_tensor("vT", [(btot // 16) * P, 2048], f16,
                          kind="ExternalInput")
    dwrelT_d = nc.dram_tensor("dwrelT", [P, btot], bf, kind="ExternalInput")
    invN_d = nc.dram_tensor("invN", [P, nw], f32, kind="ExternalInput")
    xTloc_d = nc.dram_tensor("xTloc", [DIN, npad], bf, kind="ExternalInput")
    hasrow_d = nc.dram_tensor("hasrow", [1, npad], bf, kind="ExternalInput")
    bex_d = nc.dram_tensor("bex", [1, DOUT], bf, kind="ExternalInput")
    mw1_sd_d = nc.dram_tensor("mw1_sd", [2 * DIN, 2 * DOUT], f16,
                              kind="ExternalInput")
    mb1_d = nc.dram_tensor("mb1", [2 * DOUT, 1], f32, kind="ExternalInput")
    mw2_d = nc.dram_tensor("mw2", [2 * DOUT, DOUT], bf, kind="ExternalInput")
    uw1_d = nc.dram_tensor("uw1", [DIN + DOUT, DOUT], bf,
                           kind="ExternalInput")
    ub1_d = nc.dram_tensor("ub1", [DOUT, 1], f32, kind="ExternalInput")
    uw2_d = nc.dram_tensor("uw2", [DOUT, DOUT], bf, kind="ExternalInput")
    ub2_d = nc.dram_tensor("ub2", [DOUT, 1], f32, kind="ExternalInput")
    lng_d = nc.dram_tensor("lng", [P, DOUT], f32, kind="ExternalInput")
    lnb_d = nc.dram_tensor("lnb", [P, DOUT], f32, kind="ExternalInput")
    iota512_d = nc.dram_tensor("iota512", [P, 512], bf,
                               kind="ExternalInput")
    ident_d = nc.dram_tensor("ident", [P, P], bf, kind="ExternalInput")
    out_d = nc.dram_tensor("out", [npad, DOUT], f32, kind="ExternalOutput")

    AX = mybir.AxisListType
    OP = mybir.AluOpType
    ACT = mybir.ActivationFunctionType

    with tile.TileContext(nc) as tc:
        with (
            tc.tile_pool(name="const", bufs=1) as cpool,
            tc.tile_pool(name="gath", bufs=4) as gpool,
            tc.tile_pool(name="work", bufs=4) as wpool,
            tc.tile_pool(name="oh", bufs=6) as opool,
            tc.tile_pool(name="pt", bufs=2, space="PSUM") as pt_pool,
            tc.tile_pool(name="ph", bufs=2, space="PSUM") as ph_pool,
            tc.tile_pool(name="pm", bufs=2, space="PSUM") as pm_pool,
            tc.tile_pool(name="pa", bufs=2, space="PSUM") as pa_pool,
        ):
            def cload(dram, shape, dtype=f32):
                t = cpool.tile(shape, dtype, name=dram.name + "_t")
                nc.sync.dma_start(out=t[:], in_=dram[:])
                return t

            iota512_t = cload(iota512_d, [P, 512], bf)
            ident_t = cload(ident_d, [P, P], bf)
            mw1_sd_t = cload(mw1_sd_d, [2 * DIN, 2 * DOUT], f16)
            mb1_t = cload(mb1_d, [2 * DOUT, 1])
            mw2_t = cload(mw2_d, [2 * DOUT, DOUT], bf)
            bex_t = cload(bex_d, [1, DOUT], bf)
            dwrelT_t = cload(dwrelT_d, [P, btot], bf)
            invN_t = cload(invN_d, [P, nw])
            hasrow_t = cload(hasrow_d, [1, npad], bf)
            uw1_t = cload(uw1_d, [DIN + DOUT, DOUT], bf)
            ub1_t = cload(ub1_d, [DOUT, 1])
            uw2_t = cload(uw2_d, [DOUT, DOUT], bf)
            ub2_t = cload(ub2_d, [DOUT, 1])
            lng_t = cload(lng_d, [P, DOUT])
            lnb_t = cload(lnb_d, [P, DOUT])

            eps_t = cpool.tile([P, 1], f32, name="eps_t")
            nc.vector.memset(eps_t[:], 1e-5)

            # per-4-window update-MLP input chunks:
            # rows 0:64 = aggT, rows 64:128 = xT
            UT = 512
            nchunk = (npad + UT - 1) // UT
            upd_c = []
            for k in range(nchunk):
                cw = min(UT, npad - k * UT)
                t = cpool.tile([P, cw], bf, name=f"upd_c{k}")
                nc.sync.dma_start(out=t[DOUT:P, :],
                                  in_=xTloc_d[:, k * UT:k * UT + cw])
                upd_c.append(t)

            # LN intermediates parked per chunk; sqrt batched at the end
            zc_all = [cpool.tile([P, 4 * DOUT], f32, name=f"zc_all{k}")
                      for k in range(nchunk)]
            red2_all = cpool.tile([P, 4 * nchunk], f32, name="red2_all")

            pa_cur = {}
            upd_done = [False] * nchunk

            def emit_upd(k):
                # ---- update MLP + LayerNorm for node chunk k ----
                upd_done[k] = True
                u0 = k * UT
                cw = min(UT, npad - u0)
                nj = cw // P
                upd_in = upd_c[k]
                pu = ph_pool.tile([P, 512], f32, tag="ph", name=f"pu_{u0}")
                nc.tensor.matmul(pu[0:DOUT, 0:cw], uw1_t[:],
                                 upd_in[:, 0:cw], start=True, stop=False)
                nc.tensor.matmul(pu[0:DOUT, 0:cw], bex_t[:],
                                 hasrow_t[:, u0:u0 + cw], start=False,
                                 stop=True, skip_group_check=True)
                uh_sb = wpool.tile([DOUT, UT], bf, tag="uh", name=f"uh_{u0}")
                nc.scalar.activation(out=uh_sb[:, 0:cw],
                                     in_=pu[0:DOUT, 0:cw],
                                     func=ACT.Silu, bias=ub1_t[:, 0:1])
                pz = pt_pool.tile([P, 512], f32, tag="pt", name=f"pz_{u0}")
                nc.tensor.matmul(pz[0:DOUT, 0:cw], uw2_t[:], uh_sb[:, 0:cw],
                                 start=True, stop=True)
                zT_sb = wpool.tile([DOUT, UT], bf, tag="zT", name=f"zT_{u0}")
                nc.scalar.activation(out=zT_sb[:, 0:cw], in_=pz[0:DOUT, 0:cw],
                                     func=ACT.Identity, bias=ub2_t[:, 0:1])

                pz2 = pm_pool.tile([P, 4 * DOUT], bf, tag="pm",
                                   name=f"pz2_{u0}")
                for j in range(nj):
                    nc.tensor.transpose(
                        out=pz2[:, j * DOUT:(j + 1) * DOUT],
                        in_=zT_sb[:, j * P:(j + 1) * P],
                        identity=ident_t[0:DOUT, 0:DOUT])
                # LN phase A: mean-center + variance sum; sqrt deferred
                zc = zc_all[k]
                red = wpool.tile([P, 4], f32, tag="red", name=f"red_{u0}")
                z3 = pz2[:, 0:nj * DOUT].rearrange("p (j d) -> p j d", d=DOUT)
                nc.vector.tensor_reduce(out=red[:, 0:nj], in_=z3, axis=AX.X,
                                        op=OP.add)
                nc.vector.tensor_scalar_mul(red[:, 0:nj], red[:, 0:nj],
                                            -1.0 / DOUT)
                zc3 = zc[:, 0:nj * DOUT].rearrange("p (j d) -> p j d", d=DOUT)
                nc.vector.tensor_tensor(
                    out=zc3, in0=z3,
                    in1=red[:, 0:nj, None].to_broadcast([P, nj, DOUT]),
                    op=OP.add)
                sq = wpool.tile([P, 4 * DOUT], f32, tag="sq", name=f"sq_{u0}")
                sq3 = sq[:, 0:nj * DOUT].rearrange("p (j d) -> p j d", d=DOUT)
                nc.vector.tensor_tensor(out=sq3, in0=zc3, in1=zc3, op=OP.mult)
                nc.vector.tensor_reduce(out=red2_all[:, 4 * k:4 * k + nj],
                                        in_=sq3, axis=AX.X, op=OP.add)

            def emit_ln_final():
                # batched sqrt + reciprocal, then scale/affine/store per chunk
                sd = cpool.tile([P, 4 * nchunk], f32, name="sd_all")
                nc.scalar.activation(out=sd[:], in_=red2_all[:],
                                     func=ACT.Sqrt, scale=1.0 / DOUT,
                                     bias=eps_t[:, 0:1])
                rs = cpool.tile([P, 4 * nchunk], f32, name="rs_all")
                nc.vector.reciprocal(out=rs[:], in_=sd[:])
                for k in range(nchunk):
                    u0 = k * UT
                    cw = min(UT, npad - u0)
                    nj = cw // P
                    zc = zc_all[k]
                    zc3 = zc[:, 0:nj * DOUT].rearrange("p (j d) -> p j d",
                                                       d=DOUT)
                    zn = wpool.tile([P, 4 * DOUT], f32, tag="zn",
                                    name=f"zn_{u0}")
                    zn3 = zn[:, 0:nj * DOUT].rearrange("p (j d) -> p j d",
                                                       d=DOUT)
                    nc.vector.tensor_tensor(
                        out=zn3, in0=zc3,
                        in1=rs[:, 4 * k:4 * k + nj, None]
                            .to_broadcast([P, nj, DOUT]),
                        op=OP.mult)
                    nc.vector.scalar_tensor_tensor(
                        out=zn3, in0=zn3, scalar=1.0,
                        in1=lng_t[:, None, :].to_broadcast([P, nj, DOUT]),
                        op0=OP.mult, op1=OP.mult)
                    nc.vector.tensor_tensor(
                        out=zn3, in0=zn3,
                        in1=lnb_t[:, None, :].to_broadcast([P, nj, DOUT]),
                        op=OP.add)
                    od = out_d[u0:u0 + cw, :].rearrange(
                        "(j p) d -> p j d", p=P)
                    zn3o = zn[:, 0:nj * DOUT].rearrange(
                        "p (j d) -> p j d", d=DOUT)
                    nc.sync.dma_start(out=od, in_=zn3o)

            iota3 = iota512_t[:].rearrange("p (j c) -> p j c", c=P)

            for g0 in range(0, btot, 4):
                st = g0 // 4
                if st % 4 == 0:
                    xp4 = gpool.tile([P, 2048], f16, tag="xp",
                                     name=f"xp_{g0}")
                    nc.sync.dma_start(
                        out=xp4[:],
                        in_=vT_d[(st // 4) * P:(st // 4 + 1) * P, :])
                xp = xp4[:, (st % 4) * 512:(st % 4 + 1) * 512]

                oh4 = opool.tile([P, 4, P], bf, tag="oh", name=f"oh_{g0}")
                nc.vector.tensor_tensor(
                    out=oh4[:],
                    in0=iota3,
                    in1=dwrelT_t[:, g0:g0 + 4, None].to_broadcast([P, 4, P]),
                    op=OP.is_equal)

                ph = ph_pool.tile([P, 512], f32, tag="ph", name=f"ph_{g0}")
                nc.tensor.matmul(ph[:], mw1_sd_t[:], xp,
                                 start=True, stop=True)
                hT_sb = wpool.tile([P, 512], bf, tag="hT", name=f"hT_{g0}")
                nc.scalar.activation(out=hT_sb[:], in_=ph[:],
                                     func=ACT.Silu, bias=mb1_t[:, 0:1])

                pm = pm_pool.tile([P, 4 * DOUT], f32, tag="pm",
                                  name=f"pm_{g0}")
                for j in range(4):
                    nc.tensor.matmul(pm[:, j * DOUT:(j + 1) * DOUT],
                                     hT_sb[:, j * P:(j + 1) * P],
                                     mw2_t[:], start=True, stop=True)
                msg_sb = wpool.tile([P, 4 * DOUT], bf, tag="msg",
                                    name=f"msg_{g0}")
                if st % 2 == 0:
                    nc.vector.tensor_copy(out=msg_sb[:], in_=pm[:])
                else:
                    nc.scalar.copy(out=msg_sb[:], in_=pm[:])

                for j in range(4):
                    g = g0 + j
                    w = block_window[g]
                    if g == wfirst[w]:
                        pa_cur[w] = pa_pool.tile([P, DOUT], f32, tag="pa",
                                                 name=f"pa_w{w}")
                    nc.tensor.matmul(
                        pa_cur[w][:],
                        oh4[:, j, :],
                        msg_sb[:, j * DOUT:(j + 1) * DOUT],
                        start=(g == wfirst[w]), stop=(g == wlast[w]),
                        skip_group_check=True)
                    if g != wlast[w]:
                        continue
                    # ---- window flush ----
                    s_nT = wpool.tile([P, DOUT], bf, tag="snt",
                                      name=f"snt_{w}")
                    nc.vector.tensor_tensor(
                        out=s_nT[:], in0=pa_cur[w][:],
                        in1=invN_t[:, w:w + 1].to_broadcast([P, DOUT]),
                        op=OP.mult)
                    del pa_cur[w]
                    agg_ps = pt_pool.tile([P, P], bf, tag="pt",
                                          name=f"agg_{w}")
                    nc.tensor.transpose(agg_ps[0:DOUT, :], s_nT[:],
                                        ident_t[:])
                    kc = w // 4
                    uc = slice((w % 4) * P, (w % 4 + 1) * P)
                    nc.scalar.copy(out=upd_c[kc][0:DOUT, uc],
                                   in_=agg_ps[0:DOUT, :])
                    if w == min(4 * (w // 4) + 4, nw) - 1:
                        emit_upd(w // 4)

            for k in range(nchunk):
                if not upd_done[k]:
                    emit_upd(k)
            emit_ln_final()

    nc.compile()
    return nc


# ---------------------------------------------------------------------------
# Entry point
# ---------------------------------------------------------------------------

last_results = None


def kernel(x, edge_index, edge_vec, edge_len,
           mw1, mb1, mw2, mb2, uw1, ub1, uw2, ub2, ln_g, ln_b):
    global last_results
    import os
    from concourse.bass_utils import run_bass_kernel_spmd

    struct, in_maps = _build_host_data(
        x, edge_index, edge_len, mw1, mb1, mw2, mb2,
        uw1, ub1, uw2, ub2, ln_g, ln_b)

    key = (struct["n"], struct["btot"], struct["bws"])
    if key not in _prog_cache:
        _prog_cache[key] = _build_program(struct)
    nc = _prog_cache[key]

    kw = {}
    if os.environ.get("K_TRACE", ""):
        try:
            import profile_shim
            profile_shim.install()
        except ImportError:
            pass
        kw = dict(trace=True, trace_cores=list(range(NCORES)),
                  tmpdir="/tmp/ntff_out")
    res = run_bass_kernel_spmd(nc, in_maps, core_ids=list(range(NCORES)), **kw)
    last_results = res
    nloc = struct["nloc"]
    out = np.concatenate([res.results[c]["out"][:nloc] for c in range(NCORES)],
                         axis=0)
    return out.astype(np.float32)
